# revision 50
# baseline (speedup 1.0000x reference)
"""GQA attention kernel for Trainium2, sharded over 8 NeuronCores.

Problem (hardcoded): B=4, S=1024, HID=2048, 16 query heads, 4 KV heads,
head_dim=128, RoPE (base 10000), causal softmax, O-projection.

Sharding: core c handles (batch b = c//2, head-half = c%2): 8 query heads,
2 KV heads, and the matching column/row shards of Wq/Wk/Wv/Wo. Each core
produces a partial O-projection output [S, HID] (plus a small partial-O
side output); the host sums the per-batch partials.

v4 design (evolved from the 286us v2 baseline via perfetto analysis;
measures ~200-204us warm / ~235us when the chip P0-downclocks):
- scores->exp in 5 ragged PSUM groups per head: one [128,<=1024] two-bank
  scalar ACTIVATE per group instead of 12 narrow ones (the 352-cycle/instr
  ACT overhead was half the scalar time, and PSUM recycling had been
  gating the score matmuls at scalar speed).
- softmax denominator accumulated ON the PE: ones-matmuls into a
  dedicated [1,1024] PSUM bank pair, interleaved right after each exp
  group; the idle Vector engine pre-folds kt-pairs (6 bf16 adds) so the
  PE only contracts 2560 of the 4608 columns. This replaced the v2/v3
  serial pre-sum chains whose cross-engine latency caused 5-9us PE
  stalls per head and HAM re-throttling to 1.2GHz.
- Q-projection in fp8 e4m3 with DoubleRow (2 k-tiles per matmul, 2
  MACs/cell/cycle): Wq is host-prescaled by 64 (folded into the exp
  scale); the fp8 x copy is cast on-device from the bf16 stream by the
  idle DVE during pass 1 (saves 2MB of the DMA-bound prologue). Softmax
  compresses the quantization error: rel err 0.0123 vs the 0.02 gate
  (bf16 was 0.0026).
  K/V/O stay bf16 - their error hits the output linearly and the max-err
  tails do not fit the budget.
- software pipeline: Q-proj of head h+2 and score groups G0/G1 of head
  h+1 are interleaved into head h, so the exp chain (the scalar-engine
  critical path) starts ~4us early and the PE never waits on it.
- RoPE via a P64 permutation matmul into the ps_dn bank pair + 3 DVE ops
  (sign folded into the sin table). An SBUF->SBUF DMA rotate variant was
  faster on paper but kept serializing behind Tile's counting-semaphore
  aliasing across DMA queues.
- partial O-projection: heads 6/7 have no Q-proj left to pipeline, so 10
  output tiles accumulate their heads-0-5 contribution there, shipped via
  a second DRAM output that the host adds in; the final O-proj phase then
  only needs 2 matmuls for those tiles (-15us of serial tail).
- PSUM budget (8 banks): ps_sc [128,1024]x2 for score groups + ctx,
  ps_q [128,512]x2 for Q-proj/partial-O, ps_dn [128,1024]x1 for
  denominator + rope scratch. Ring orders are chosen so no allocation
  waits on later-program-order work (deadlock/stall-free by construction).
- DMA: 3 queues; first pass-1 matmul needs only the first transfer of
  each queue; x (bf16+fp8) streams ahead of pass-1 consumption; Wo and
  the tail Wq tiles prefetch mid-head-loop; output stores round-robin
  sync/gpsimd with an 8-bank O-proj PSUM ring.

Per-core layouts (partition dim first, 128 everywhere):
  xT   [128, kt(16), s(1024)]   xT[p,kt,s] = x[s, 128kt+p]          bf16
  xkp8 x re-cast to fp8 on-device, kt-pair-major for DoubleRow     fp8e4
  wq8  [128, h(8)*kt(16), d(128)]  64*Wq                            fp8e4
  wk/wv[128, kt(16), c(2)*d(128)]                                   bf16
  wo   [128, h(8), e(2048)]     wo[p,h,e] = Wo[1024*half+128h+p, e] bf16
  qT/kT[128d, s]   roped in T orientation
  v    [128s, st(8)*c(2)*d(128)]  natural, for ctx stationary
  eS   [128k, ragged q spans]   exp(scores^T) bf16, 4608 cols
  ctxT [128d, s] per head       normalized context, feeds O-proj
"""
import math
from contextlib import ExitStack

import numpy as np
import ml_dtypes

import concourse.bass as bass
import concourse.bacc as bacc
import concourse.tile as tile
from concourse import mybir
from concourse.bass_utils import run_bass_kernel_spmd

F32 = mybir.dt.float32
BF16 = mybir.dt.bfloat16
F8E4 = mybir.dt.float8e4
QSCALE = 64.0  # host premultiplies Wq by this before fp8; folded into exp

B, S, HID = 4, 1024, 2048
NH, NKV, D = 16, 4, 128
HPC = 8          # query heads per core
KVPC = 2         # kv heads per core
SCALE = 1.0 / math.sqrt(D)
NKT = HID // 128  # 16 contraction tiles
NST = S // 128    # 8 sequence tiles
KVD = KVPC * D    # 256

# eS ragged packing: block for key-tile kt covers q in [128kt, S), stored at
# eS col ES_OFF[kt] + (q - 128kt).
ES_OFF = []
_o = 0
for _kt in range(NST):
    ES_OFF.append(_o)
    _o += S - 128 * _kt
ES_W = _o  # 4608

# exp groups: eS col ranges, each <=1024 wide (one [128,1024] 2-bank PSUM
# tile + one wide ACTIVATE). Pieces never cross a 512 (bank) boundary.
GB = [0, 1024, 2048, 3072, 4096, ES_W]
NG = 5


def _group_pieces(g):
    glo, ghi = GB[g], GB[g + 1]
    out = []
    for kt in range(NST):
        blo = ES_OFF[kt]
        bhi = blo + (S - 128 * kt)
        lo, hi = max(blo, glo), min(bhi, ghi)
        if lo >= hi:
            continue
        p = lo
        while p < hi:
            nxt = min(hi, (p // 512 + 1) * 512)
            out.append((kt, 128 * kt + (p - blo), 128 * kt + (nxt - blo),
                        p - glo))
            p = nxt
    return out


GROUP_PIECES = [_group_pieces(g) for g in range(NG)]
# O-proj tiles whose heads 0-5 partial is precomputed during heads 6/7
PRE_TILES = [(st, ec) for st in range(5) for ec in range(2)]
# which kts have their (diagonal-masked) block start inside group g
DIAG_G = [[kt for kt in range(NST) if GB[g] <= ES_OFF[kt] < GB[g + 1]]
          for g in range(NG)]

# denominator accumulation pieces: per kt, the q-span [128kt, S) split at the
# j boundary (q=512, the dn PSUM bank edge). Each piece is issued after the
# exp group that contains its last eS column (groups complete in order).
# kt0 pieces carry start=True (kt0 spans all q, so it initializes both
# banks); the final writer of each bank carries stop=True.
DN_G = [[] for _ in range(NG)]
for _kt in range(NST):
    _spans = [(128 * _kt, 512), (512, S)] if 128 * _kt < 512 else \
        [(128 * _kt, S)]
    for (_q0, _q1) in _spans:
        _lastcol = ES_OFF[_kt] + (_q1 - 1) - 128 * _kt
        _g = next(g for g in range(NG) if _lastcol < GB[g + 1])
        _stop = (_kt == 3 and _q1 == 512) or (_kt == 7)
        DN_G[_g].append((_kt, _q0, _q1, _kt == 0, _stop))


def build_kernel():
    nc = bacc.Bacc(None)
    xT = nc.dram_tensor("xT", [128, NKT * S], BF16, kind="ExternalInput")
    # fp8 Wq for the Q projection (DoubleRow: 2 k-tiles per pass); the fp8
    # x copy is cast on-device from the bf16 stream (saves 2MB of the
    # DMA-bound prologue)
    wq8 = nc.dram_tensor("wq8", [128, HPC * NKT, D], F8E4,
                         kind="ExternalInput")
    wk = nc.dram_tensor("wk", [128, NKT * KVD], BF16, kind="ExternalInput")
    wv = nc.dram_tensor("wv", [128, NKT * KVD], BF16, kind="ExternalInput")
    wo = nc.dram_tensor("wo", [128, HPC * HID], BF16, kind="ExternalInput")
    # tables bundled: cosT(1024) | sinTs(1024) | p64(128, unused) | dmask(128)
    tbl = nc.dram_tensor("tbl", [128, 2304], BF16, kind="ExternalInput")
    out = nc.dram_tensor("out", [S, HID], F32, kind="ExternalOutput")
    # partial O-proj (heads 0-5) for 10 tiles, computed during heads 6/7 in
    # the PE slots freed by the ended Q-proj pipeline; host adds this into
    # out[0:640, 0:1024]
    out2 = nc.dram_tensor("out2", [5 * 128, 2 * 512], F32,
                          kind="ExternalOutput")

    with tile.TileContext(nc) as tc, ExitStack() as top:
        const = top.enter_context(tc.tile_pool(name="const", bufs=1))
        xk_pool = top.enter_context(tc.tile_pool(name="xk", bufs=1))
        wkv_pool = top.enter_context(tc.tile_pool(name="wkv", bufs=1))
        kT_pool = top.enter_context(tc.tile_pool(name="kT", bufs=1))
        v_pool = top.enter_context(tc.tile_pool(name="v", bufs=1))
        ctxT_pool = top.enter_context(tc.tile_pool(name="ctxT", bufs=1))
        wq_pool = top.enter_context(tc.tile_pool(name="wq", bufs=4))
        qT_pool = top.enter_context(tc.tile_pool(name="qT", bufs=3))
        eS_pool = top.enter_context(tc.tile_pool(name="eS", bufs=2))
        rc_pool = top.enter_context(tc.tile_pool(name="rc", bufs=2))
        tmp_pool = top.enter_context(tc.tile_pool(name="tmp", bufs=2))
        wo_pool = top.enter_context(tc.tile_pool(name="wo", bufs=1))
        out_pool = top.enter_context(tc.tile_pool(name="outp", bufs=6))

        # ---- input DMA over the 3 DMA-capable queues (sync/gpsimd/scalar);
        # first pass-1 matmul needs only wk_h0 (scalar q), wv_h0 (gpsimd q),
        # xkp0 (sync q) — each is the first transfer on its queue ----------
        wk_sb = wkv_pool.tile([128, NKT * KVD], BF16, tag="wk")
        wv_sb = wkv_pool.tile([128, NKT * KVD], BF16, tag="wv")
        HK = NKT * KVD // 2
        nc.scalar.dma_start(wk_sb[:, 0:HK], wk[:, 0:HK])
        nc.gpsimd.dma_start(wv_sb[:, 0:HK], wv[:, 0:HK])

        xkp = [xk_pool.tile([128, 2 * S], BF16, tag=f"xkp{g}", name=f"xkp{g}")
               for g in range(NKT // 2)]
        # fp8 x copy for Q-proj (needed from qproj(0) right after pass 1):
        # cast on-device from the bf16 chunks on the otherwise-idle DVE
        xkp8 = [xk_pool.tile([128, 2, S], F8E4, tag=f"xkp8_{g}",
                             name=f"xkp8_{g}") for g in range(NKT // 2)]
        for g in range(NKT // 2):
            eng = nc.sync if g % 2 == 0 else nc.gpsimd
            if g == 0:
                # halves: the first pass-1 matmul only needs kt=0
                eng.dma_start(xkp[0][:, 0:S], xT[:, 0:S])
                eng.dma_start(xkp[0][:, S:2 * S], xT[:, S:2 * S])
            else:
                eng.dma_start(xkp[g][:], xT[:, 2 * g * S:(2 * g + 2) * S])
        for g in range(NKT // 2):
            nc.vector.tensor_copy(xkp8[g][:], xkp[g][:])

        # keep sync/gpsimd dedicated to x during pass 1; everything else
        # (needed from pass 2 onward) streams on the scalar queue in
        # first-use order
        wq_sb = {}
        for h in range(4):
            wq_sb[h] = wq_pool.tile([128, NKT, D], F8E4, tag="wqh",
                                    name=f"wqh{h}")
        nc.scalar.dma_start(wk_sb[:, HK:], wk[:, HK:])
        nc.scalar.dma_start(wv_sb[:, HK:], wv[:, HK:])
        nc.scalar.dma_start(wq_sb[0][:], wq8[:, 0:NKT, :])
        tbl_sb = const.tile([128, 2304], BF16)
        nc.scalar.dma_start(tbl_sb[:], tbl[:])
        nc.scalar.dma_start(wq_sb[1][:], wq8[:, NKT:2 * NKT, :])
        nc.scalar.dma_start(wq_sb[2][:], wq8[:, 2 * NKT:3 * NKT, :])
        nc.scalar.dma_start(wq_sb[3][:], wq8[:, 3 * NKT:4 * NKT, :])

        def xkc(kt, a, b):
            g, r = kt // 2, kt % 2
            return xkp[g][:, r * S + a:r * S + b]

        cosT_sb = tbl_sb[:, 0:S]
        sinTs_sb = tbl_sb[:, S:2 * S]
        p64_sb = tbl_sb[:, 2 * S:2 * S + D]
        dmask_sb = tbl_sb[:, 2 * S + D:2 * S + 2 * D]

        ones_bf = const.tile([128, 1], BF16)
        nc.vector.memset(ones_bf[:], 1.0)

        ctxT = [ctxT_pool.tile([D, S], BF16, tag=f"ctxT{h}", name=f"ctxT{h}")
                for h in range(HPC)]
        kT = [kT_pool.tile([D, S], BF16, tag=f"kT{c}", name=f"kT{c}")
              for c in range(KVPC)]
        v_sb = v_pool.tile([128, NST * KVD], BF16)

        # ---- pass 1: stream kt chunks once -> all of K proj + V st0-3 ----
        with ExitStack() as pro:
            psP = pro.enter_context(tc.tile_pool(name="psP", bufs=1,
                                                 space="PSUM"))
            kps = {}
            for c in range(KVPC):
                for sc in range(2):
                    kps[(c, sc)] = psP.tile([128, 512], F32, tag=f"kp{c}{sc}",
                                            name=f"kp{c}{sc}")
            vps = [psP.tile([128, 512], F32, tag=f"vp{st}", name=f"vp{st}")
                   for st in range(4)]
            for kt in range(NKT):
                for c in range(KVPC):
                    for sc in range(2):
                        nc.tensor.matmul(
                            kps[(c, sc)][:],
                            wk_sb[:, kt * KVD + c * D:kt * KVD + (c + 1) * D],
                            xkc(kt, sc * 512, (sc + 1) * 512),
                            start=(kt == 0), stop=(kt == NKT - 1))
                for st in range(4):
                    nc.tensor.matmul(
                        vps[st][:, 0:KVD], xkc(kt, st * 128, (st + 1) * 128),
                        wv_sb[:, kt * KVD:(kt + 1) * KVD],
                        start=(kt == 0), stop=(kt == NKT - 1))
            # evacuations split across vector+scalar so the psP-pool
            # release barrier (gating pass-2's first matmuls) clears ~2x
            # sooner
            for c in range(KVPC):
                for sc in range(2):
                    nc.vector.tensor_copy(kT[c][:, sc * 512:(sc + 1) * 512],
                                          kps[(c, sc)][:])
            for st in range(4):
                nc.scalar.copy(v_sb[:, st * KVD:(st + 1) * KVD],
                               vps[st][:, 0:KVD])

        # ---- pass 2 + head loop PSUM pools: exactly 8 banks --------------
        with ExitStack() as mid:
            ps_sc = mid.enter_context(tc.tile_pool(name="ps_sc", bufs=2,
                                                   space="PSUM"))
            ps_q = mid.enter_context(tc.tile_pool(name="ps_q", bufs=2,
                                                  space="PSUM"))
            ps_dn = mid.enter_context(tc.tile_pool(name="ps_dn", bufs=1,
                                                   space="PSUM"))

            def rope_inplace(dst, rope_id):
                """RoPE in T orientation on a [128, S] bf16 tile in place.
                Partition rotate-by-64 via a P64 perm matmul into the ps_dn
                bank pair (idle between denominator uses; sign of the
                rotate is folded into sinTs), then 3 DVE ops."""
                rot = ps_dn.tile([128, 1024], F32, tag="dn",
                                 name=f"rot{rope_id}")
                for sc in range(2):
                    nc.tensor.matmul(rot[:, sc * 512:(sc + 1) * 512], p64_sb,
                                     dst[:, sc * 512:(sc + 1) * 512],
                                     start=True, stop=True)
                tmp = tmp_pool.tile([128, S], BF16, tag="ropetmp",
                                    name=f"rtmp{rope_id}")
                nc.vector.tensor_mul(tmp[:], rot[:], sinTs_sb)
                nc.vector.tensor_mul(dst[:], dst[:], cosT_sb)
                nc.vector.tensor_add(dst[:], dst[:], tmp[:])

            # pass 2: K rope -> fp8 Q proj heads 0/1 -> V st4-7, so the
            # qT copy + rope latency hides under the V matmuls. K ropes get
            # dedicated rot tiles + the scalar DMA queue (free here) so no
            # ring WAR chains them to the qT ropes.
            for c in range(KVPC):
                rope_inplace(kT[c], f"k{c}")

            qT = {}

            def qproj_psum(hq, sc, p0, p1, ps):
                # fp8 DoubleRow: each matmul contracts a PAIR of k-tiles
                # (2*128 rows) at 2 MACs/cell/cycle
                for p in range(p0, p1):
                    nc.tensor.matmul(
                        ps[:], wq_sb[hq][:, 2 * p:2 * p + 2, :],
                        xkp8[p][:, :, sc * 512:(sc + 1) * 512],
                        start=(p == 0), stop=(p == NKT // 2 - 1),
                        perf_mode=mybir.MatmulPerfMode.DoubleRow)

            for hq in range(2):
                qT[hq] = qT_pool.tile([D, S], BF16, tag="qTh",
                                      name=f"qT{hq}")
                for sc in range(2):
                    ps = ps_q.tile([128, 512], F32, tag="ps_q",
                                   name=f"q{hq}s{sc}")
                    qproj_psum(hq, sc, 0, NKT // 2, ps)
                    nc.scalar.copy(qT[hq][:, sc * 512:(sc + 1) * 512], ps[:])
                rope_inplace(qT[hq], f"q{hq}")


            # ---- head loop with 2-group scores lookahead: G0/G1 of head
            # h+1 are issued near the end of head h so its exp (the scalar
            # critical path) gets a head start --------------------------------
            eS_t = {}

            def scores_group(h, g):
                eS = eS_t[h]
                c = h // (HPC // KVPC)
                w = GB[g + 1] - GB[g]
                ps = ps_sc.tile([128, 1024], F32, tag="ps_sc",
                                name=f"sc{h}_{g}")
                for (kt, q0, q1, rel) in GROUP_PIECES[g]:
                    nc.tensor.matmul(
                        ps[:, rel:rel + q1 - q0],
                        kT[c][:, kt * 128:(kt + 1) * 128],
                        qT[h][:, q0:q1], start=True, stop=True)
                nc.scalar.activation(
                    eS[:, GB[g]:GB[g + 1]], ps[:, 0:w],
                    mybir.ActivationFunctionType.Exp, scale=SCALE / QSCALE)
                for kt in DIAG_G[g]:
                    off = ES_OFF[kt]
                    nc.vector.tensor_mul(eS[:, off:off + 128],
                                         eS[:, off:off + 128], dmask_sb)

            dnf_t = {}

            def begin_head(h):
                eS_t[h] = eS_pool.tile([128, ES_W], BF16, tag="eS",
                                       name=f"eS{h}")
                dnf_t[h] = tmp_pool.tile([128, 2048], BF16, tag="dnf",
                                         name=f"dnf{h}")
                scores_group(h, 0)
                scores_group(h, 1)

            begin_head(0)
            nc.vector.tensor_add(dnf_t[0][:, 0:384], eS_t[0][:, 128:512],
                                 eS_t[0][:, 1024:1408])
            nc.vector.tensor_add(dnf_t[0][:, 512:1024], eS_t[0][:, 512:1024],
                                 eS_t[0][:, 1408:1920])
            for st in range(4, NST):
                ps = ps_q.tile([128, 512], F32, tag="ps_q", name=f"vq{st}")
                for kt in range(NKT):
                    nc.tensor.matmul(
                        ps[:, 0:KVD], xkc(kt, st * 128, (st + 1) * 128),
                        wv_sb[:, kt * KVD:(kt + 1) * KVD],
                        start=(kt == 0), stop=(kt == NKT - 1))
                nc.vector.tensor_copy(v_sb[:, st * KVD:(st + 1) * KVD],
                                      ps[:, 0:KVD])
            wo_sb = None
            for h in range(HPC):
                c = h // (HPC // KVPC)  # local kv head
                hq = h + 2              # head whose Q-proj we compute now
                eS = eS_t[h]

                if hq < HPC:
                    qT[hq] = qT_pool.tile([D, S], BF16, tag="qTh",
                                          name=f"qT{hq}")
                    psq0 = ps_q.tile([128, 512], F32, tag="ps_q",
                                     name=f"q{hq}s0")
                dn = ps_dn.tile([128, 1024], F32, tag="dn", name=f"dn{h}")

                def op_partial(k):
                    # heads 6/7: partial O-proj (heads 0-5) in the PE slots
                    # the Q-proj pipeline no longer needs
                    st, ec = PRE_TILES[(h - 6) * 5 + k]
                    po2 = ps_q.tile([128, 512], F32, tag="ps_q",
                                    name=f"op{st}_{ec}")
                    for hh in range(6):
                        nc.tensor.matmul(
                            po2[:], ctxT[hh][:, st * 128:(st + 1) * 128],
                            wo_sb[:, hh * HID + ec * 512:
                                  hh * HID + (ec + 1) * 512],
                            start=(hh == 0), stop=(hh == 5))
                    ot2 = out_pool.tile([128, 512], F32, tag="ot",
                                        name=f"ot2_{st}_{ec}")
                    dst = out2[st * 128:(st + 1) * 128,
                               ec * 512:(ec + 1) * 512]
                    nc.vector.tensor_copy(ot2[:], po2[:])
                    if k % 2:
                        nc.gpsimd.dma_start(dst, ot2[:])
                    else:
                        nc.sync.dma_start(dst, ot2[:])

                dnf = dnf_t[h]

                def dn_mm(q0, q1, src, st_, sp_):
                    # denominator partial sums on the PE: ones-matmul over
                    # raw eS pieces or DVE-prefolded kt-pairs (dnf), PSUM-
                    # accumulated into dn[0, q0:q1] (~halves the PE columns)
                    nc.tensor.matmul(dn[:1, q0:q1], ones_bf[:], src,
                                     start=st_, stop=sp_)

                scores_group(h, 2)
                if hq < HPC:
                    qproj_psum(hq, 0, 0, 4, psq0)
                nc.vector.tensor_add(dnf[:, 384:512], eS[:, 2048:2176],
                                     eS[:, 2688:2816])
                dn_mm(0, 128, eS[:, 0:128], True, False)
                dn_mm(128, 512, dnf[:, 0:384], True, False)
                dn_mm(512, 1024, dnf[:, 512:1024], True, False)
                if hq >= HPC:
                    op_partial(0)
                scores_group(h, 3)
                if hq < HPC:
                    qproj_psum(hq, 0, 4, 6, psq0)
                else:
                    op_partial(1)
                nc.vector.tensor_add(dnf[:, 1024:1536], eS[:, 2176:2688],
                                     eS[:, 2816:3328])
                dn_mm(256, 384, eS[:, 1920:2048], False, False)
                dn_mm(384, 512, dnf[:, 384:512], False, True)
                scores_group(h, 4)
                if hq < HPC:
                    qproj_psum(hq, 0, 6, 8, psq0)
                    psq1 = ps_q.tile([128, 512], F32, tag="ps_q",
                                     name=f"q{hq}s1")
                    qproj_psum(hq, 1, 0, 4, psq1)
                else:
                    op_partial(2)
                if h + 1 < HPC:
                    # lookahead scores G0/G1 of head h+1 issued here so the
                    # exps complete well before head h+1's first score mms
                    begin_head(h + 1)
                nc.vector.tensor_add(dnf[:, 1536:1920], eS[:, 3456:3840],
                                     eS[:, 3840:4224])
                nc.vector.tensor_add(dnf[:, 1920:2048], eS[:, 4352:4480],
                                     eS[:, 4480:4608])
                dn_mm(512, 1024, dnf[:, 1024:1536], False, False)
                dn_mm(512, 640, eS[:, 3328:3456], False, False)
                dn_mm(640, 1024, dnf[:, 1536:1920], False, False)
                dn_mm(768, 896, eS[:, 4224:4352], False, False)
                dn_mm(896, 1024, dnf[:, 1920:2048], False, True)
                if hq < HPC:
                    qproj_psum(hq, 1, 4, 8, psq1)
                else:
                    op_partial(3)
                if hq < HPC:
                    # qT copies issued after the lookahead exps so the
                    # scalar exp stream is never interrupted mid-head
                    nc.scalar.copy(qT[hq][:, 0:512], psq0[:])
                    nc.scalar.copy(qT[hq][:, 512:1024], psq1[:])

                rc = rc_pool.tile([1, S], F32, tag="rc", name=f"rc{h}")
                nc.vector.reciprocal_approx_fast(rc[:1, 0:512],
                                                 dn[:1, 0:512])
                rb0 = tmp_pool.tile([128, 512], F32, tag="rbtmp",
                                    name=f"rb0_{h}")
                nc.gpsimd.partition_broadcast(rb0[:], rc[:1, 0:512])
                nc.vector.reciprocal_approx_fast(rc[:1, 512:1024],
                                                 dn[:1, 512:1024])
                rb1 = tmp_pool.tile([128, 512], F32, tag="rbtmp",
                                    name=f"rb1_{h}")
                nc.gpsimd.partition_broadcast(rb1[:], rc[:1, 512:1024])

                if hq >= HPC:
                    op_partial(4)

                # ctx matmuls (ragged accumulate) into one [128,1024] ps_sc
                # tile (keeps the ps_q ring free of the normalize chain)
                pcx = ps_sc.tile([128, 1024], F32, tag="ps_sc",
                                 name=f"pc_{h}")
                for j in range(2):
                    kts = [kt for kt in range(NST)
                           if max(128 * kt, j * 512) < (j + 1) * 512]
                    for kt in kts:
                        qlo = 128 * kt
                        lo = max(qlo, j * 512)
                        hi = (j + 1) * 512
                        nc.tensor.matmul(
                            pcx[:, lo:hi],
                            v_sb[:, kt * KVD + c * D:kt * KVD + (c + 1) * D],
                            eS[:, ES_OFF[kt] + lo - qlo:ES_OFF[kt] + hi - qlo],
                            start=(kt == kts[0]), stop=(kt == kts[-1]))
                nc.vector.tensor_mul(ctxT[h][:, 0:512], pcx[:, 0:512],
                                     rb0[:])
                nc.vector.tensor_mul(ctxT[h][:, 512:1024], pcx[:, 512:1024],
                                     rb1[:])

                if hq < HPC:
                    rope_inplace(qT[hq], f"q{hq}")
                if h + 1 < HPC:
                    eS1, dnf1 = eS_t[h + 1], dnf_t[h + 1]
                    nc.vector.tensor_add(dnf1[:, 0:384], eS1[:, 128:512],
                                         eS1[:, 1024:1408])
                    nc.vector.tensor_add(dnf1[:, 512:1024],
                                         eS1[:, 512:1024],
                                         eS1[:, 1408:1920])

                # staged weight prefetches
                if h < 4 and h + 4 < HPC:
                    wq_sb[h + 4] = wq_pool.tile([128, NKT, D], F8E4,
                                                tag="wqh", name=f"wqh{h+4}")
                    nc.gpsimd.dma_start(
                        wq_sb[h + 4][:],
                        wq8[:, (h + 4) * NKT:(h + 5) * NKT, :])
                if h == 2:  # wo arrives while attention still running
                    wo_sb = wo_pool.tile([128, HPC * HID], BF16)
                    HW2 = HPC * HID // 2
                    nc.sync.dma_start(wo_sb[:, 0:HW2], wo[:, 0:HW2])
                    nc.gpsimd.dma_start(wo_sb[:, HW2:], wo[:, HW2:])

        # ---- O projection: own 8-bank PSUM pool, deep pipeline -----------
        with ExitStack() as fin:
            psO = fin.enter_context(tc.tile_pool(name="psO", bufs=8,
                                                 space="PSUM"))
            pre = set(PRE_TILES)
            for st in range(NST):
                for ec in range(HID // 512):
                    po = psO.tile([128, 512], F32, tag="po",
                                  name=f"po{st}_{ec}")
                    h0 = 6 if (st, ec) in pre else 0
                    for h in range(h0, HPC):
                        nc.tensor.matmul(
                            po[:], ctxT[h][:, st * 128:(st + 1) * 128],
                            wo_sb[:, h * HID + ec * 512:h * HID + (ec + 1) * 512],
                            start=(h == h0), stop=(h == HPC - 1))
                    ot = out_pool.tile([128, 512], F32, tag="ot")
                    if (st * 4 + ec) % 2:
                        nc.scalar.copy(ot[:], po[:])
                        nc.gpsimd.dma_start(
                            out[st * 128:(st + 1) * 128,
                                ec * 512:(ec + 1) * 512], ot[:])
                    else:
                        nc.vector.tensor_copy(ot[:], po[:])
                        nc.sync.dma_start(
                            out[st * 128:(st + 1) * 128,
                                ec * 512:(ec + 1) * 512], ot[:])
    nc.finalize()
    return nc


def host_prep(hidden_states, Wq, Wk, Wv, Wo):
    """Pre-transpose/cast/relayout all inputs on the host (bf16 + fp8)."""
    bf = ml_dtypes.bfloat16
    f8 = ml_dtypes.float8_e4m3fn
    xTs = []
    for b in range(B):
        t = hidden_states[b].T.reshape(NKT, 128, S).transpose(1, 0, 2)
        xTs.append(np.ascontiguousarray(t.astype(bf)).reshape(128, NKT * S))
    halves = []
    for hf in range(2):
        wqh = Wq[:, 1024 * hf:1024 * (hf + 1)].reshape(NKT, 128, HPC, D)
        wqh = np.ascontiguousarray(
            np.clip(wqh.transpose(1, 2, 0, 3) * QSCALE, -240, 240)
            .astype(f8)).reshape(128, HPC * NKT * D)
        wkh = Wk[:, KVD * hf:KVD * (hf + 1)].reshape(NKT, 128, KVD)
        wkh = np.ascontiguousarray(
            wkh.transpose(1, 0, 2).astype(bf)).reshape(128, NKT * KVD)
        wvh = Wv[:, KVD * hf:KVD * (hf + 1)].reshape(NKT, 128, KVD)
        wvh = np.ascontiguousarray(
            wvh.transpose(1, 0, 2).astype(bf)).reshape(128, NKT * KVD)
        woh = Wo[1024 * hf:1024 * (hf + 1), :].reshape(HPC, 128, HID)
        woh = np.ascontiguousarray(
            woh.transpose(1, 0, 2).astype(bf)).reshape(128, HPC * HID)
        halves.append((wqh, wkh, wvh, woh))

    inv_freq = 1.0 / (10000.0 ** (np.arange(0, D, 2, dtype=np.float64) / D))
    t = np.arange(S, dtype=np.float64)
    freqs = np.outer(t, inv_freq)
    emb = np.concatenate([freqs, freqs], -1)
    cosT = np.cos(emb).T
    sinTs_f = np.sin(emb).T.copy()
    sinTs_f[:64] *= -1.0
    p64 = np.zeros((D, D), dtype=np.float64)
    for d in range(D):
        p64[d, (d + 64) % D] = 1.0
    dmask = np.triu(np.ones((128, 128), dtype=np.float64))
    tbl = np.ascontiguousarray(
        np.concatenate([cosT, sinTs_f, p64, dmask], axis=1)).astype(bf)
    return xTs, halves, tbl


_CACHE = {}


def kernel(hidden_states, Wq, Wk, Wv, Wo, _trace=False, _tmpdir=None):
    hidden_states = np.ascontiguousarray(hidden_states, dtype=np.float32)
    Wq = np.ascontiguousarray(Wq, dtype=np.float32)
    Wk = np.ascontiguousarray(Wk, dtype=np.float32)
    Wv = np.ascontiguousarray(Wv, dtype=np.float32)
    Wo = np.ascontiguousarray(Wo, dtype=np.float32)

    if "nc" not in _CACHE:
        _CACHE["nc"] = build_kernel()
    nc = _CACHE["nc"]
    xTs, halves, tbl = host_prep(hidden_states, Wq, Wk, Wv, Wo)

    in_maps = []
    for cid in range(8):
        b, hf = cid // 2, cid % 2
        wqh, wkh, wvh, woh = halves[hf]
        in_maps.append({
            "xT": xTs[b], "wq8": wqh, "wk": wkh, "wv": wvh,
            "wo": woh, "tbl": tbl,
        })
    res = run_bass_kernel_spmd(nc, in_maps, list(range(8)),
                               trace=_trace, tmpdir=_tmpdir)
    out = np.zeros((B, S, HID), dtype=np.float32)
    for cid in range(8):
        out[cid // 2] += res.results[cid]["out"]
        out[cid // 2][0:640, 0:1024] += res.results[cid]["out2"]
    if _trace:
        return out, res
    return out


# revision 51
# speedup vs baseline: 1.1277x; 1.1277x over previous
"""GQA attention kernel for Trainium2, sharded over 8 NeuronCores.

Problem (hardcoded): B=4, S=1024, HID=2048, 16 query heads, 4 KV heads,
head_dim=128, RoPE (base 10000), causal softmax, O-projection.

Sharding: core c handles (batch b = c//2, head-half = c%2): 8 query heads,
2 KV heads, and the matching column/row shards of Wq/Wk/Wv/Wo. Each core
produces a partial O-projection output [S, HID] (plus a small partial-O
side output); the host sums the per-batch partials.

v4 design (evolved from the 286us v2 baseline via perfetto analysis;
measures ~200-204us warm / ~235us when the chip P0-downclocks):
- scores->exp in 5 ragged PSUM groups per head: one [128,<=1024] two-bank
  scalar ACTIVATE per group instead of 12 narrow ones (the 352-cycle/instr
  ACT overhead was half the scalar time, and PSUM recycling had been
  gating the score matmuls at scalar speed).
- softmax denominator accumulated ON the PE: ones-matmuls into a
  dedicated [1,1024] PSUM bank pair, interleaved right after each exp
  group; the idle Vector engine pre-folds kt-pairs (6 bf16 adds) so the
  PE only contracts 2560 of the 4608 columns. This replaced the v2/v3
  serial pre-sum chains whose cross-engine latency caused 5-9us PE
  stalls per head and HAM re-throttling to 1.2GHz.
- Q-projection in fp8 e4m3 with DoubleRow (2 k-tiles per matmul, 2
  MACs/cell/cycle): Wq is host-prescaled by 64 (folded into the exp
  scale); the fp8 x copy is cast on-device from the bf16 stream by the
  idle DVE during pass 1 (saves 2MB of the DMA-bound prologue). Softmax
  compresses the quantization error: rel err 0.0123 vs the 0.02 gate
  (bf16 was 0.0026).
  K/V/O stay bf16 - their error hits the output linearly and the max-err
  tails do not fit the budget.
- software pipeline: Q-proj of head h+2 and score groups G0/G1 of head
  h+1 are interleaved into head h, so the exp chain (the scalar-engine
  critical path) starts ~4us early and the PE never waits on it.
- RoPE via a P64 permutation matmul into the ps_dn bank pair + 3 DVE ops
  (sign folded into the sin table). An SBUF->SBUF DMA rotate variant was
  faster on paper but kept serializing behind Tile's counting-semaphore
  aliasing across DMA queues.
- partial O-projection: heads 6/7 have no Q-proj left to pipeline, so 10
  output tiles accumulate their heads-0-5 contribution there, shipped via
  a second DRAM output that the host adds in; the final O-proj phase then
  only needs 2 matmuls for those tiles (-15us of serial tail).
- PSUM budget (8 banks): ps_sc [128,1024]x2 for score groups + ctx,
  ps_q [128,512]x2 for Q-proj/partial-O, ps_dn [128,1024]x1 for
  denominator + rope scratch. Ring orders are chosen so no allocation
  waits on later-program-order work (deadlock/stall-free by construction).
- DMA: 3 queues; first pass-1 matmul needs only the first transfer of
  each queue; x (bf16+fp8) streams ahead of pass-1 consumption; Wo and
  the tail Wq tiles prefetch mid-head-loop; output stores round-robin
  sync/gpsimd with an 8-bank O-proj PSUM ring.

Per-core layouts (partition dim first, 128 everywhere):
  xT   [128, kt(16), s(1024)]   xT[p,kt,s] = x[s, 128kt+p]          bf16
  xkp8 x re-cast to fp8 on-device, kt-pair-major for DoubleRow     fp8e4
  wq8  [128, h(8)*kt(16), d(128)]  64*Wq                            fp8e4
  wk/wv[128, kt(16), c(2)*d(128)]                                   bf16
  wo   [128, h(8), e(2048)]     wo[p,h,e] = Wo[1024*half+128h+p, e] bf16
  qT/kT[128d, s]   roped in T orientation
  v    [128s, st(8)*c(2)*d(128)]  natural, for ctx stationary
  eS   [128k, ragged q spans]   exp(scores^T) bf16, 4608 cols
  ctxT [128d, s] per head       normalized context, feeds O-proj
"""
import math
from contextlib import ExitStack

import numpy as np
import ml_dtypes

import concourse.bass as bass
import concourse.bacc as bacc
import concourse.tile as tile
from concourse import mybir
from concourse.bass_utils import run_bass_kernel_spmd

F32 = mybir.dt.float32
BF16 = mybir.dt.bfloat16
F8E4 = mybir.dt.float8e4
QSCALE = 64.0  # host premultiplies Wq by this before fp8; folded into exp

B, S, HID = 4, 1024, 2048
NH, NKV, D = 16, 4, 128
HPC = 8          # query heads per core
KVPC = 2         # kv heads per core
SCALE = 1.0 / math.sqrt(D)
NKT = HID // 128  # 16 contraction tiles
NST = S // 128    # 8 sequence tiles
KVD = KVPC * D    # 256

# eS ragged packing: block for key-tile kt covers q in [128kt, S), stored at
# eS col ES_OFF[kt] + (q - 128kt).
ES_OFF = []
_o = 0
for _kt in range(NST):
    ES_OFF.append(_o)
    _o += S - 128 * _kt
ES_W = _o  # 4608

# exp groups: eS col ranges, each <=1024 wide (one [128,1024] 2-bank PSUM
# tile + one wide ACTIVATE). Pieces never cross a 512 (bank) boundary.
GB = [0, 1024, 2048, 3072, 4096, ES_W]
NG = 5


def _group_pieces(g):
    glo, ghi = GB[g], GB[g + 1]
    out = []
    for kt in range(NST):
        blo = ES_OFF[kt]
        bhi = blo + (S - 128 * kt)
        lo, hi = max(blo, glo), min(bhi, ghi)
        if lo >= hi:
            continue
        p = lo
        while p < hi:
            nxt = min(hi, (p // 512 + 1) * 512)
            out.append((kt, 128 * kt + (p - blo), 128 * kt + (nxt - blo),
                        p - glo))
            p = nxt
    return out


GROUP_PIECES = [_group_pieces(g) for g in range(NG)]
# O-proj tiles whose heads 0-5 partial is precomputed during heads 6/7
PRE_TILES = [(st, ec) for st in range(5) for ec in range(2)]
# which kts have their (diagonal-masked) block start inside group g
DIAG_G = [[kt for kt in range(NST) if GB[g] <= ES_OFF[kt] < GB[g + 1]]
          for g in range(NG)]

# denominator accumulation pieces: per kt, the q-span [128kt, S) split at the
# j boundary (q=512, the dn PSUM bank edge). Each piece is issued after the
# exp group that contains its last eS column (groups complete in order).
# kt0 pieces carry start=True (kt0 spans all q, so it initializes both
# banks); the final writer of each bank carries stop=True.
DN_G = [[] for _ in range(NG)]
for _kt in range(NST):
    _spans = [(128 * _kt, 512), (512, S)] if 128 * _kt < 512 else \
        [(128 * _kt, S)]
    for (_q0, _q1) in _spans:
        _lastcol = ES_OFF[_kt] + (_q1 - 1) - 128 * _kt
        _g = next(g for g in range(NG) if _lastcol < GB[g + 1])
        _stop = (_kt == 3 and _q1 == 512) or (_kt == 7)
        DN_G[_g].append((_kt, _q0, _q1, _kt == 0, _stop))


def build_kernel():
    nc = bacc.Bacc(None)
    xT = nc.dram_tensor("xT", [128, NKT * S], BF16, kind="ExternalInput")
    # fp8 Wq for the Q projection (DoubleRow: 2 k-tiles per pass); the fp8
    # x copy is cast on-device from the bf16 stream (saves 2MB of the
    # DMA-bound prologue)
    wq8 = nc.dram_tensor("wq8", [128, HPC * NKT, D], F8E4,
                         kind="ExternalInput")
    wk = nc.dram_tensor("wk", [128, NKT * KVD], BF16, kind="ExternalInput")
    wv = nc.dram_tensor("wv", [128, NKT * KVD], BF16, kind="ExternalInput")
    wo = nc.dram_tensor("wo", [128, HPC * HID], BF16, kind="ExternalInput")
    # tables bundled: cosT(1024) | sinTs(1024) | p64(128, unused) | dmask(128)
    tbl = nc.dram_tensor("tbl", [128, 2304], BF16, kind="ExternalInput")
    out = nc.dram_tensor("out", [S, HID], F32, kind="ExternalOutput")
    # partial O-proj (heads 0-5) for 10 tiles, computed during heads 6/7 in
    # the PE slots freed by the ended Q-proj pipeline; host adds this into
    # out[0:640, 0:1024]
    out2 = nc.dram_tensor("out2", [5 * 128, 2 * 512], F32,
                          kind="ExternalOutput")

    with tile.TileContext(nc) as tc, ExitStack() as top:
        const = top.enter_context(tc.tile_pool(name="const", bufs=1))
        xk_pool = top.enter_context(tc.tile_pool(name="xk", bufs=1))
        wkv_pool = top.enter_context(tc.tile_pool(name="wkv", bufs=1))
        kT_pool = top.enter_context(tc.tile_pool(name="kT", bufs=1))
        v_pool = top.enter_context(tc.tile_pool(name="v", bufs=1))
        ctxT_pool = top.enter_context(tc.tile_pool(name="ctxT", bufs=1))
        wq_pool = top.enter_context(tc.tile_pool(name="wq", bufs=4))
        qT_pool = top.enter_context(tc.tile_pool(name="qT", bufs=3))
        eS_pool = top.enter_context(tc.tile_pool(name="eS", bufs=2))
        rc_pool = top.enter_context(tc.tile_pool(name="rc", bufs=2))
        tmp_pool = top.enter_context(tc.tile_pool(name="tmp", bufs=2))
        wo_pool = top.enter_context(tc.tile_pool(name="wo", bufs=1))
        out_pool = top.enter_context(tc.tile_pool(name="outp", bufs=6))

        # ---- input DMA over the 3 DMA-capable queues (sync/gpsimd/scalar);
        # first pass-1 matmul needs only wk_h0 (scalar q), wv_h0 (gpsimd q),
        # xkp0 (sync q) — each is the first transfer on its queue ----------
        wk_sb = wkv_pool.tile([128, NKT * KVD], BF16, tag="wk")
        wv_sb = wkv_pool.tile([128, NKT * KVD], BF16, tag="wv")
        HK = NKT * KVD // 2
        nc.scalar.dma_start(wk_sb[:, 0:HK // 2], wk[:, 0:HK // 2])
        nc.scalar.dma_start(wk_sb[:, HK // 2:HK], wk[:, HK // 2:HK])
        nc.gpsimd.dma_start(wv_sb[:, 0:HK], wv[:, 0:HK])

        xkp = [xk_pool.tile([128, 2 * S], BF16, tag=f"xkp{g}", name=f"xkp{g}")
               for g in range(NKT // 2)]
        # fp8 x copy for Q-proj (needed from qproj(0) right after pass 1):
        # cast on-device from the bf16 chunks on the otherwise-idle DVE
        xkp8 = [xk_pool.tile([128, 2, S], F8E4, tag=f"xkp8_{g}",
                             name=f"xkp8_{g}") for g in range(NKT // 2)]
        for g in range(NKT // 2):
            eng = nc.sync if g % 2 == 0 else nc.gpsimd
            if g == 0:
                # halves: the first pass-1 matmul only needs kt=0
                eng.dma_start(xkp[0][:, 0:S], xT[:, 0:S])
                eng.dma_start(xkp[0][:, S:2 * S], xT[:, S:2 * S])
            else:
                eng.dma_start(xkp[g][:], xT[:, 2 * g * S:(2 * g + 2) * S])
        for g in range(NKT // 2):
            nc.vector.tensor_copy(xkp8[g][:], xkp[g][:])

        # keep sync/gpsimd dedicated to x during pass 1; everything else
        # (needed from pass 2 onward) streams on the scalar queue in
        # first-use order
        wq_sb = {}
        for h in range(4):
            wq_sb[h] = wq_pool.tile([128, NKT, D], F8E4, tag="wqh",
                                    name=f"wqh{h}")
        nc.scalar.dma_start(wk_sb[:, HK:], wk[:, HK:])
        nc.scalar.dma_start(wv_sb[:, HK:], wv[:, HK:])
        nc.scalar.dma_start(wq_sb[0][:], wq8[:, 0:NKT, :])
        tbl_sb = const.tile([128, 2304], BF16)
        nc.scalar.dma_start(tbl_sb[:], tbl[:])
        nc.scalar.dma_start(wq_sb[1][:], wq8[:, NKT:2 * NKT, :])
        nc.scalar.dma_start(wq_sb[2][:], wq8[:, 2 * NKT:3 * NKT, :])
        nc.scalar.dma_start(wq_sb[3][:], wq8[:, 3 * NKT:4 * NKT, :])

        def xkc(kt, a, b):
            g, r = kt // 2, kt % 2
            return xkp[g][:, r * S + a:r * S + b]

        cosT_sb = tbl_sb[:, 0:S]
        sinTs_sb = tbl_sb[:, S:2 * S]
        p64_sb = tbl_sb[:, 2 * S:2 * S + D]
        dmask_sb = tbl_sb[:, 2 * S + D:2 * S + 2 * D]

        ones_bf = const.tile([128, 1], BF16)
        nc.vector.memset(ones_bf[:], 1.0)

        ctxT = [ctxT_pool.tile([D, S], BF16, tag=f"ctxT{h}", name=f"ctxT{h}")
                for h in range(HPC)]
        kT = [kT_pool.tile([D, S], BF16, tag=f"kT{c}", name=f"kT{c}")
              for c in range(KVPC)]
        v_sb = v_pool.tile([128, NST * KVD], BF16)

        # ---- pass 1: stream kt chunks once -> all of K proj + V st0-3 ----
        with ExitStack() as pro:
            psP = pro.enter_context(tc.tile_pool(name="psP", bufs=1,
                                                 space="PSUM"))
            kps = {}
            for c in range(KVPC):
                for sc in range(2):
                    kps[(c, sc)] = psP.tile([128, 512], F32, tag=f"kp{c}{sc}",
                                            name=f"kp{c}{sc}")
            vps = [psP.tile([128, 512], F32, tag=f"vp{st}", name=f"vp{st}")
                   for st in range(4)]
            for kt in range(NKT):
                for c in range(KVPC):
                    for sc in range(2):
                        nc.tensor.matmul(
                            kps[(c, sc)][:],
                            wk_sb[:, kt * KVD + c * D:kt * KVD + (c + 1) * D],
                            xkc(kt, sc * 512, (sc + 1) * 512),
                            start=(kt == 0), stop=(kt == NKT - 1))
                for st in range(4):
                    nc.tensor.matmul(
                        vps[st][:, 0:KVD], xkc(kt, st * 128, (st + 1) * 128),
                        wv_sb[:, kt * KVD:(kt + 1) * KVD],
                        start=(kt == 0), stop=(kt == NKT - 1))
            # evacuations split across vector+scalar so the psP-pool
            # release barrier (gating pass-2's first matmuls) clears ~2x
            # sooner
            for c in range(KVPC):
                for sc in range(2):
                    nc.vector.tensor_copy(kT[c][:, sc * 512:(sc + 1) * 512],
                                          kps[(c, sc)][:])
            for st in range(4):
                nc.scalar.copy(v_sb[:, st * KVD:(st + 1) * KVD],
                               vps[st][:, 0:KVD])

        # ---- pass 2 + head loop PSUM pools: exactly 8 banks --------------
        with ExitStack() as mid:
            ps_sc = mid.enter_context(tc.tile_pool(name="ps_sc", bufs=2,
                                                   space="PSUM"))
            ps_q = mid.enter_context(tc.tile_pool(name="ps_q", bufs=2,
                                                  space="PSUM"))
            ps_dn = mid.enter_context(tc.tile_pool(name="ps_dn", bufs=1,
                                                   space="PSUM"))

            def rope_inplace(dst, rope_id):
                """RoPE in T orientation on a [128, S] bf16 tile in place.
                Partition rotate-by-64 via a P64 perm matmul into the ps_dn
                bank pair (idle between denominator uses; sign of the
                rotate is folded into sinTs), then 3 DVE ops."""
                rot = ps_dn.tile([128, 1024], F32, tag="dn",
                                 name=f"rot{rope_id}")
                for sc in range(2):
                    nc.tensor.matmul(rot[:, sc * 512:(sc + 1) * 512], p64_sb,
                                     dst[:, sc * 512:(sc + 1) * 512],
                                     start=True, stop=True)
                tmp = tmp_pool.tile([128, S], BF16, tag="ropetmp",
                                    name=f"rtmp{rope_id}")
                nc.vector.tensor_mul(tmp[:], rot[:], sinTs_sb)
                nc.vector.tensor_mul(dst[:], dst[:], cosT_sb)
                nc.vector.tensor_add(dst[:], dst[:], tmp[:])

            # pass 2: K rope -> fp8 Q proj heads 0/1 -> V st4-7, so the
            # qT copy + rope latency hides under the V matmuls. K ropes get
            # dedicated rot tiles + the scalar DMA queue (free here) so no
            # ring WAR chains them to the qT ropes.
            for c in range(KVPC):
                rope_inplace(kT[c], f"k{c}")

            qT = {}

            def qproj_psum(hq, sc, p0, p1, ps):
                # fp8 DoubleRow: each matmul contracts a PAIR of k-tiles
                # (2*128 rows) at 2 MACs/cell/cycle
                for p in range(p0, p1):
                    nc.tensor.matmul(
                        ps[:], wq_sb[hq][:, 2 * p:2 * p + 2, :],
                        xkp8[p][:, :, sc * 512:(sc + 1) * 512],
                        start=(p == 0), stop=(p == NKT // 2 - 1),
                        perf_mode=mybir.MatmulPerfMode.DoubleRow)

            for hq in range(2):
                qT[hq] = qT_pool.tile([D, S], BF16, tag="qTh",
                                      name=f"qT{hq}")
                for sc in range(2):
                    ps = ps_q.tile([128, 512], F32, tag="ps_q",
                                   name=f"q{hq}s{sc}")
                    qproj_psum(hq, sc, 0, NKT // 2, ps)
                    nc.scalar.copy(qT[hq][:, sc * 512:(sc + 1) * 512], ps[:])
                rope_inplace(qT[hq], f"q{hq}")


            # ---- head loop with 2-group scores lookahead: G0/G1 of head
            # h+1 are issued near the end of head h so its exp (the scalar
            # critical path) gets a head start --------------------------------
            eS_t = {}

            def scores_group(h, g):
                eS = eS_t[h]
                c = h // (HPC // KVPC)
                w = GB[g + 1] - GB[g]
                ps = ps_sc.tile([128, 1024], F32, tag="ps_sc",
                                name=f"sc{h}_{g}")
                for (kt, q0, q1, rel) in GROUP_PIECES[g]:
                    nc.tensor.matmul(
                        ps[:, rel:rel + q1 - q0],
                        kT[c][:, kt * 128:(kt + 1) * 128],
                        qT[h][:, q0:q1], start=True, stop=True)
                nc.scalar.activation(
                    eS[:, GB[g]:GB[g + 1]], ps[:, 0:w],
                    mybir.ActivationFunctionType.Exp, scale=SCALE / QSCALE)
                for kt in DIAG_G[g]:
                    off = ES_OFF[kt]
                    nc.vector.tensor_mul(eS[:, off:off + 128],
                                         eS[:, off:off + 128], dmask_sb)

            dnf_t = {}

            def begin_head(h):
                eS_t[h] = eS_pool.tile([128, ES_W], BF16, tag="eS",
                                       name=f"eS{h}")
                dnf_t[h] = tmp_pool.tile([128, 2048], BF16, tag="dnf",
                                         name=f"dnf{h}")
                scores_group(h, 0)
                scores_group(h, 1)

            begin_head(0)
            nc.vector.tensor_add(dnf_t[0][:, 0:384], eS_t[0][:, 128:512],
                                 eS_t[0][:, 1024:1408])
            nc.vector.tensor_add(dnf_t[0][:, 512:1024], eS_t[0][:, 512:1024],
                                 eS_t[0][:, 1408:1920])
            for st in range(4, NST):
                ps = ps_q.tile([128, 512], F32, tag="ps_q", name=f"vq{st}")
                for kt in range(NKT):
                    nc.tensor.matmul(
                        ps[:, 0:KVD], xkc(kt, st * 128, (st + 1) * 128),
                        wv_sb[:, kt * KVD:(kt + 1) * KVD],
                        start=(kt == 0), stop=(kt == NKT - 1))
                nc.vector.tensor_copy(v_sb[:, st * KVD:(st + 1) * KVD],
                                      ps[:, 0:KVD])
            wo_sb = None
            for h in range(HPC):
                c = h // (HPC // KVPC)  # local kv head
                hq = h + 2              # head whose Q-proj we compute now
                eS = eS_t[h]

                if hq < HPC:
                    qT[hq] = qT_pool.tile([D, S], BF16, tag="qTh",
                                          name=f"qT{hq}")
                    psq0 = ps_q.tile([128, 512], F32, tag="ps_q",
                                     name=f"q{hq}s0")
                dn = ps_dn.tile([128, 1024], F32, tag="dn", name=f"dn{h}")

                def op_partial(k):
                    # heads 6/7: partial O-proj (heads 0-5) in the PE slots
                    # the Q-proj pipeline no longer needs
                    st, ec = PRE_TILES[(h - 6) * 5 + k]
                    po2 = ps_q.tile([128, 512], F32, tag="ps_q",
                                    name=f"op{st}_{ec}")
                    for hh in range(6):
                        nc.tensor.matmul(
                            po2[:], ctxT[hh][:, st * 128:(st + 1) * 128],
                            wo_sb[:, hh * HID + ec * 512:
                                  hh * HID + (ec + 1) * 512],
                            start=(hh == 0), stop=(hh == 5))
                    ot2 = out_pool.tile([128, 512], F32, tag="ot",
                                        name=f"ot2_{st}_{ec}")
                    dst = out2[st * 128:(st + 1) * 128,
                               ec * 512:(ec + 1) * 512]
                    nc.vector.tensor_copy(ot2[:], po2[:])
                    if k % 2:
                        nc.gpsimd.dma_start(dst, ot2[:])
                    else:
                        nc.sync.dma_start(dst, ot2[:])

                dnf = dnf_t[h]

                def dn_mm(q0, q1, src, st_, sp_):
                    # denominator partial sums on the PE: ones-matmul over
                    # raw eS pieces or DVE-prefolded kt-pairs (dnf), PSUM-
                    # accumulated into dn[0, q0:q1] (~halves the PE columns)
                    nc.tensor.matmul(dn[:1, q0:q1], ones_bf[:], src,
                                     start=st_, stop=sp_)

                scores_group(h, 2)
                if hq < HPC:
                    qproj_psum(hq, 0, 0, 4, psq0)
                nc.vector.tensor_add(dnf[:, 384:512], eS[:, 2048:2176],
                                     eS[:, 2688:2816])
                dn_mm(0, 128, eS[:, 0:128], True, False)
                dn_mm(128, 512, dnf[:, 0:384], True, False)
                dn_mm(512, 1024, dnf[:, 512:1024], True, False)
                if hq >= HPC:
                    op_partial(0)
                scores_group(h, 3)
                if hq < HPC:
                    qproj_psum(hq, 0, 4, 6, psq0)
                else:
                    op_partial(1)
                nc.vector.tensor_add(dnf[:, 1024:1536], eS[:, 2176:2688],
                                     eS[:, 2816:3328])
                dn_mm(256, 384, eS[:, 1920:2048], False, False)
                dn_mm(384, 512, dnf[:, 384:512], False, True)
                scores_group(h, 4)
                if hq < HPC:
                    qproj_psum(hq, 0, 6, 8, psq0)
                    psq1 = ps_q.tile([128, 512], F32, tag="ps_q",
                                     name=f"q{hq}s1")
                    qproj_psum(hq, 1, 0, 4, psq1)
                else:
                    op_partial(2)
                if h + 1 < HPC:
                    # lookahead scores G0/G1 of head h+1 issued here so the
                    # exps complete well before head h+1's first score mms
                    begin_head(h + 1)
                nc.vector.tensor_add(dnf[:, 1536:1920], eS[:, 3456:3840],
                                     eS[:, 3840:4224])
                nc.vector.tensor_add(dnf[:, 1920:2048], eS[:, 4352:4480],
                                     eS[:, 4480:4608])
                dn_mm(512, 1024, dnf[:, 1024:1536], False, False)
                dn_mm(512, 640, eS[:, 3328:3456], False, False)
                dn_mm(640, 1024, dnf[:, 1536:1920], False, False)
                dn_mm(768, 896, eS[:, 4224:4352], False, False)
                dn_mm(896, 1024, dnf[:, 1920:2048], False, True)
                if hq < HPC:
                    qproj_psum(hq, 1, 4, 8, psq1)
                else:
                    op_partial(3)
                if hq < HPC:
                    # qT copies issued after the lookahead exps so the
                    # scalar exp stream is never interrupted mid-head
                    nc.scalar.copy(qT[hq][:, 0:512], psq0[:])
                    nc.scalar.copy(qT[hq][:, 512:1024], psq1[:])

                rc = rc_pool.tile([1, S], F32, tag="rc", name=f"rc{h}")
                nc.vector.reciprocal_approx_fast(rc[:1, 0:512],
                                                 dn[:1, 0:512])
                rb0 = tmp_pool.tile([128, 512], F32, tag="rbtmp",
                                    name=f"rb0_{h}")
                nc.gpsimd.partition_broadcast(rb0[:], rc[:1, 0:512])
                nc.vector.reciprocal_approx_fast(rc[:1, 512:1024],
                                                 dn[:1, 512:1024])
                rb1 = tmp_pool.tile([128, 512], F32, tag="rbtmp",
                                    name=f"rb1_{h}")
                nc.gpsimd.partition_broadcast(rb1[:], rc[:1, 512:1024])

                if hq >= HPC:
                    op_partial(4)

                # ctx matmuls (ragged accumulate) into one [128,1024] ps_sc
                # tile (keeps the ps_q ring free of the normalize chain)
                pcx = ps_sc.tile([128, 1024], F32, tag="ps_sc",
                                 name=f"pc_{h}")
                for j in range(2):
                    kts = [kt for kt in range(NST)
                           if max(128 * kt, j * 512) < (j + 1) * 512]
                    for kt in kts:
                        qlo = 128 * kt
                        lo = max(qlo, j * 512)
                        hi = (j + 1) * 512
                        nc.tensor.matmul(
                            pcx[:, lo:hi],
                            v_sb[:, kt * KVD + c * D:kt * KVD + (c + 1) * D],
                            eS[:, ES_OFF[kt] + lo - qlo:ES_OFF[kt] + hi - qlo],
                            start=(kt == kts[0]), stop=(kt == kts[-1]))
                nc.vector.tensor_mul(ctxT[h][:, 0:512], pcx[:, 0:512],
                                     rb0[:])
                nc.vector.tensor_mul(ctxT[h][:, 512:1024], pcx[:, 512:1024],
                                     rb1[:])

                if hq < HPC:
                    rope_inplace(qT[hq], f"q{hq}")
                if h + 1 < HPC:
                    eS1, dnf1 = eS_t[h + 1], dnf_t[h + 1]
                    nc.vector.tensor_add(dnf1[:, 0:384], eS1[:, 128:512],
                                         eS1[:, 1024:1408])
                    nc.vector.tensor_add(dnf1[:, 512:1024],
                                         eS1[:, 512:1024],
                                         eS1[:, 1408:1920])

                # staged weight prefetches
                if h < 4 and h + 4 < HPC:
                    wq_sb[h + 4] = wq_pool.tile([128, NKT, D], F8E4,
                                                tag="wqh", name=f"wqh{h+4}")
                    nc.gpsimd.dma_start(
                        wq_sb[h + 4][:],
                        wq8[:, (h + 4) * NKT:(h + 5) * NKT, :])
                if h == 2:  # wo arrives while attention still running
                    wo_sb = wo_pool.tile([128, HPC * HID], BF16)
                    HW2 = HPC * HID // 2
                    nc.sync.dma_start(wo_sb[:, 0:HW2], wo[:, 0:HW2])
                    nc.gpsimd.dma_start(wo_sb[:, HW2:], wo[:, HW2:])

        # ---- O projection: own 8-bank PSUM pool, deep pipeline -----------
        with ExitStack() as fin:
            psO = fin.enter_context(tc.tile_pool(name="psO", bufs=8,
                                                 space="PSUM"))
            pre = set(PRE_TILES)
            for st in range(NST):
                for ec in range(HID // 512):
                    po = psO.tile([128, 512], F32, tag="po",
                                  name=f"po{st}_{ec}")
                    h0 = 6 if (st, ec) in pre else 0
                    for h in range(h0, HPC):
                        nc.tensor.matmul(
                            po[:], ctxT[h][:, st * 128:(st + 1) * 128],
                            wo_sb[:, h * HID + ec * 512:h * HID + (ec + 1) * 512],
                            start=(h == h0), stop=(h == HPC - 1))
                    ot = out_pool.tile([128, 512], F32, tag="ot")
                    if st == NST - 1 and ec == HID // 512 - 1:
                        # final tile: halves on both engines/queues so the
                        # tail drain is as short as possible
                        nc.vector.tensor_copy(ot[:, 0:256], po[:, 0:256])
                        nc.scalar.copy(ot[:, 256:512], po[:, 256:512])
                        nc.sync.dma_start(
                            out[st * 128:(st + 1) * 128,
                                ec * 512:ec * 512 + 256], ot[:, 0:256])
                        nc.gpsimd.dma_start(
                            out[st * 128:(st + 1) * 128,
                                ec * 512 + 256:(ec + 1) * 512],
                            ot[:, 256:512])
                    elif (st * 4 + ec) % 2:
                        nc.scalar.copy(ot[:], po[:])
                        nc.gpsimd.dma_start(
                            out[st * 128:(st + 1) * 128,
                                ec * 512:(ec + 1) * 512], ot[:])
                    else:
                        nc.vector.tensor_copy(ot[:], po[:])
                        nc.sync.dma_start(
                            out[st * 128:(st + 1) * 128,
                                ec * 512:(ec + 1) * 512], ot[:])
    nc.finalize()
    return nc


def host_prep(hidden_states, Wq, Wk, Wv, Wo):
    """Pre-transpose/cast/relayout all inputs on the host (bf16 + fp8)."""
    bf = ml_dtypes.bfloat16
    f8 = ml_dtypes.float8_e4m3fn
    xTs = []
    for b in range(B):
        t = hidden_states[b].T.reshape(NKT, 128, S).transpose(1, 0, 2)
        xTs.append(np.ascontiguousarray(t.astype(bf)).reshape(128, NKT * S))
    halves = []
    for hf in range(2):
        wqh = Wq[:, 1024 * hf:1024 * (hf + 1)].reshape(NKT, 128, HPC, D)
        wqh = np.ascontiguousarray(
            np.clip(wqh.transpose(1, 2, 0, 3) * QSCALE, -240, 240)
            .astype(f8)).reshape(128, HPC * NKT * D)
        wkh = Wk[:, KVD * hf:KVD * (hf + 1)].reshape(NKT, 128, KVD)
        wkh = np.ascontiguousarray(
            wkh.transpose(1, 0, 2).astype(bf)).reshape(128, NKT * KVD)
        wvh = Wv[:, KVD * hf:KVD * (hf + 1)].reshape(NKT, 128, KVD)
        wvh = np.ascontiguousarray(
            wvh.transpose(1, 0, 2).astype(bf)).reshape(128, NKT * KVD)
        woh = Wo[1024 * hf:1024 * (hf + 1), :].reshape(HPC, 128, HID)
        woh = np.ascontiguousarray(
            woh.transpose(1, 0, 2).astype(bf)).reshape(128, HPC * HID)
        halves.append((wqh, wkh, wvh, woh))

    inv_freq = 1.0 / (10000.0 ** (np.arange(0, D, 2, dtype=np.float64) / D))
    t = np.arange(S, dtype=np.float64)
    freqs = np.outer(t, inv_freq)
    emb = np.concatenate([freqs, freqs], -1)
    cosT = np.cos(emb).T
    sinTs_f = np.sin(emb).T.copy()
    sinTs_f[:64] *= -1.0
    p64 = np.zeros((D, D), dtype=np.float64)
    for d in range(D):
        p64[d, (d + 64) % D] = 1.0
    dmask = np.triu(np.ones((128, 128), dtype=np.float64))
    tbl = np.ascontiguousarray(
        np.concatenate([cosT, sinTs_f, p64, dmask], axis=1)).astype(bf)
    return xTs, halves, tbl


_CACHE = {}


def kernel(hidden_states, Wq, Wk, Wv, Wo, _trace=False, _tmpdir=None):
    hidden_states = np.ascontiguousarray(hidden_states, dtype=np.float32)
    Wq = np.ascontiguousarray(Wq, dtype=np.float32)
    Wk = np.ascontiguousarray(Wk, dtype=np.float32)
    Wv = np.ascontiguousarray(Wv, dtype=np.float32)
    Wo = np.ascontiguousarray(Wo, dtype=np.float32)

    if "nc" not in _CACHE:
        _CACHE["nc"] = build_kernel()
    nc = _CACHE["nc"]
    xTs, halves, tbl = host_prep(hidden_states, Wq, Wk, Wv, Wo)

    in_maps = []
    for cid in range(8):
        b, hf = cid // 2, cid % 2
        wqh, wkh, wvh, woh = halves[hf]
        in_maps.append({
            "xT": xTs[b], "wq8": wqh, "wk": wkh, "wv": wvh,
            "wo": woh, "tbl": tbl,
        })
    res = run_bass_kernel_spmd(nc, in_maps, list(range(8)),
                               trace=_trace, tmpdir=_tmpdir)
    out = np.zeros((B, S, HID), dtype=np.float32)
    for cid in range(8):
        out[cid // 2] += res.results[cid]["out"]
        out[cid // 2][0:640, 0:1024] += res.results[cid]["out2"]
    if _trace:
        return out, res
    return out


# revision 53
# speedup vs baseline: 1.1898x; 1.0551x over previous
"""GQA attention kernel for Trainium2, sharded over 8 NeuronCores.

Problem (hardcoded): B=4, S=1024, HID=2048, 16 query heads, 4 KV heads,
head_dim=128, RoPE (base 10000), causal softmax, O-projection.

Sharding: core c handles (batch b = c//2, head-half = c%2): 8 query heads,
2 KV heads, and the matching column/row shards of Wq/Wk/Wv/Wo. Each core
produces a partial O-projection output [S, HID] (plus a small partial-O
side output); the host sums the per-batch partials.

v4 design (evolved from the 286us v2 baseline via perfetto analysis;
measures ~200-204us warm / ~235us when the chip P0-downclocks):
- scores->exp in 5 ragged PSUM groups per head: one [128,<=1024] two-bank
  scalar ACTIVATE per group instead of 12 narrow ones (the 352-cycle/instr
  ACT overhead was half the scalar time, and PSUM recycling had been
  gating the score matmuls at scalar speed).
- softmax denominator accumulated ON the PE: ones-matmuls into a
  dedicated [1,1024] PSUM bank pair, interleaved right after each exp
  group; the idle Vector engine pre-folds kt-pairs (6 bf16 adds) so the
  PE only contracts 2560 of the 4608 columns. This replaced the v2/v3
  serial pre-sum chains whose cross-engine latency caused 5-9us PE
  stalls per head and HAM re-throttling to 1.2GHz.
- Q-projection in fp8 e4m3 with DoubleRow (2 k-tiles per matmul, 2
  MACs/cell/cycle): Wq is host-prescaled by 64 (folded into the exp
  scale); the fp8 x copy is cast on-device from the bf16 stream by the
  idle DVE during pass 1 (saves 2MB of the DMA-bound prologue). Softmax
  compresses the quantization error: rel err 0.0123 vs the 0.02 gate
  (bf16 was 0.0026).
  K/V/O stay bf16 - their error hits the output linearly and the max-err
  tails do not fit the budget.
- software pipeline: Q-proj of head h+2 and score groups G0/G1 of head
  h+1 are interleaved into head h, so the exp chain (the scalar-engine
  critical path) starts ~4us early and the PE never waits on it.
- RoPE via a P64 permutation matmul into the ps_dn bank pair + 3 DVE ops
  (sign folded into the sin table). An SBUF->SBUF DMA rotate variant was
  faster on paper but kept serializing behind Tile's counting-semaphore
  aliasing across DMA queues.
- partial O-projection: heads 6/7 have no Q-proj left to pipeline, so 10
  output tiles accumulate their heads-0-5 contribution there, shipped via
  a second DRAM output that the host adds in; the final O-proj phase then
  only needs 2 matmuls for those tiles (-15us of serial tail).
- PSUM budget (8 banks): ps_sc [128,1024]x2 for score groups + ctx,
  ps_q [128,512]x2 for Q-proj/partial-O, ps_dn [128,1024]x1 for
  denominator + rope scratch. Ring orders are chosen so no allocation
  waits on later-program-order work (deadlock/stall-free by construction).
- DMA: 3 queues; first pass-1 matmul needs only the first transfer of
  each queue; x (bf16+fp8) streams ahead of pass-1 consumption; Wo and
  the tail Wq tiles prefetch mid-head-loop; output stores round-robin
  sync/gpsimd with an 8-bank O-proj PSUM ring.

Per-core layouts (partition dim first, 128 everywhere):
  xT   [128, kt(16), s(1024)]   xT[p,kt,s] = x[s, 128kt+p]          bf16
  xkp8 x re-cast to fp8 on-device, kt-pair-major for DoubleRow     fp8e4
  wq8  [128, h(8)*kt(16), d(128)]  64*Wq                            fp8e4
  wk/wv[128, kt(16), c(2)*d(128)]                                   bf16
  wo   [128, h(8), e(2048)]     wo[p,h,e] = Wo[1024*half+128h+p, e] bf16
  qT/kT[128d, s]   roped in T orientation
  v    [128s, st(8)*c(2)*d(128)]  natural, for ctx stationary
  eS   [128k, ragged q spans]   exp(scores^T) bf16, 4608 cols
  ctxT [128d, s] per head       normalized context, feeds O-proj
"""
import math
from contextlib import ExitStack

import numpy as np
import ml_dtypes

import concourse.bass as bass
import concourse.bacc as bacc
import concourse.tile as tile
from concourse import mybir
from concourse.bass_utils import run_bass_kernel_spmd

F32 = mybir.dt.float32
BF16 = mybir.dt.bfloat16
F8E4 = mybir.dt.float8e4
QSCALE = 64.0  # host premultiplies Wq by this before fp8; folded into exp

B, S, HID = 4, 1024, 2048
NH, NKV, D = 16, 4, 128
HPC = 8          # query heads per core
KVPC = 2         # kv heads per core
SCALE = 1.0 / math.sqrt(D)
NKT = HID // 128  # 16 contraction tiles
NST = S // 128    # 8 sequence tiles
KVD = KVPC * D    # 256

# eS ragged packing: block for key-tile kt covers q in [128kt, S), stored at
# eS col ES_OFF[kt] + (q - 128kt).
ES_OFF = []
_o = 0
for _kt in range(NST):
    ES_OFF.append(_o)
    _o += S - 128 * _kt
ES_W = _o  # 4608

# exp groups: eS col ranges, each <=1024 wide (one [128,1024] 2-bank PSUM
# tile + one wide ACTIVATE). Pieces never cross a 512 (bank) boundary.
GB = [0, 1024, 2048, 3072, 4096, ES_W]
NG = 5


def _group_pieces(g):
    glo, ghi = GB[g], GB[g + 1]
    out = []
    for kt in range(NST):
        blo = ES_OFF[kt]
        bhi = blo + (S - 128 * kt)
        lo, hi = max(blo, glo), min(bhi, ghi)
        if lo >= hi:
            continue
        p = lo
        while p < hi:
            nxt = min(hi, (p // 512 + 1) * 512)
            out.append((kt, 128 * kt + (p - blo), 128 * kt + (nxt - blo),
                        p - glo))
            p = nxt
    return out


GROUP_PIECES = [_group_pieces(g) for g in range(NG)]
# O-proj tiles whose heads 0-5 partial is precomputed during heads 6/7
PRE_TILES = [(st, ec) for st in range(5) for ec in range(2)]
# which kts have their (diagonal-masked) block start inside group g
DIAG_G = [[kt for kt in range(NST) if GB[g] <= ES_OFF[kt] < GB[g + 1]]
          for g in range(NG)]

# denominator accumulation pieces: per kt, the q-span [128kt, S) split at the
# j boundary (q=512, the dn PSUM bank edge). Each piece is issued after the
# exp group that contains its last eS column (groups complete in order).
# kt0 pieces carry start=True (kt0 spans all q, so it initializes both
# banks); the final writer of each bank carries stop=True.
DN_G = [[] for _ in range(NG)]
for _kt in range(NST):
    _spans = [(128 * _kt, 512), (512, S)] if 128 * _kt < 512 else \
        [(128 * _kt, S)]
    for (_q0, _q1) in _spans:
        _lastcol = ES_OFF[_kt] + (_q1 - 1) - 128 * _kt
        _g = next(g for g in range(NG) if _lastcol < GB[g + 1])
        _stop = (_kt == 3 and _q1 == 512) or (_kt == 7)
        DN_G[_g].append((_kt, _q0, _q1, _kt == 0, _stop))


def build_kernel():
    nc = bacc.Bacc(None)
    xT = nc.dram_tensor("xT", [128, NKT * S], BF16, kind="ExternalInput")
    # fp8 Wq for the Q projection (DoubleRow: 2 k-tiles per pass); the fp8
    # x copy is cast on-device from the bf16 stream (saves 2MB of the
    # DMA-bound prologue)
    wq8 = nc.dram_tensor("wq8", [128, HPC * NKT, D], F8E4,
                         kind="ExternalInput")
    wk = nc.dram_tensor("wk", [128, NKT * KVD], BF16, kind="ExternalInput")
    wv = nc.dram_tensor("wv", [128, NKT * KVD], BF16, kind="ExternalInput")
    wo = nc.dram_tensor("wo", [128, HPC * HID], BF16, kind="ExternalInput")
    # tables bundled: cosT(1024) | sinTs(1024) | p64(128, unused) | dmask(128)
    tbl = nc.dram_tensor("tbl", [128, 2304], BF16, kind="ExternalInput")
    out = nc.dram_tensor("out", [S, HID], F32, kind="ExternalOutput")
    # partial O-proj (heads 0-5) for 10 tiles, computed during heads 6/7 in
    # the PE slots freed by the ended Q-proj pipeline; host adds this into
    # out[0:640, 0:1024]
    out2 = nc.dram_tensor("out2", [5 * 128, 2 * 512], F32,
                          kind="ExternalOutput")

    with tile.TileContext(nc) as tc, ExitStack() as top:
        const = top.enter_context(tc.tile_pool(name="const", bufs=1))
        xk_pool = top.enter_context(tc.tile_pool(name="xk", bufs=1))
        wkv_pool = top.enter_context(tc.tile_pool(name="wkv", bufs=1))
        kT_pool = top.enter_context(tc.tile_pool(name="kT", bufs=1))
        v_pool = top.enter_context(tc.tile_pool(name="v", bufs=1))
        ctxT_pool = top.enter_context(tc.tile_pool(name="ctxT", bufs=1))
        wq_pool = top.enter_context(tc.tile_pool(name="wq", bufs=4))
        qT_pool = top.enter_context(tc.tile_pool(name="qT", bufs=3))
        eS_pool = top.enter_context(tc.tile_pool(name="eS", bufs=2))
        rc_pool = top.enter_context(tc.tile_pool(name="rc", bufs=2))
        tmp_pool = top.enter_context(tc.tile_pool(name="tmp", bufs=2))
        wo_pool = top.enter_context(tc.tile_pool(name="wo", bufs=1))
        out_pool = top.enter_context(tc.tile_pool(name="outp", bufs=6))

        # ---- input DMA over the 3 DMA-capable queues (sync/gpsimd/scalar);
        # first pass-1 matmul needs only wk_h0 (scalar q), wv_h0 (gpsimd q),
        # xkp0 (sync q) — each is the first transfer on its queue ----------
        wk_sb = wkv_pool.tile([128, NKT * KVD], BF16, tag="wk")
        wv_sb = wkv_pool.tile([128, NKT * KVD], BF16, tag="wv")
        HK = NKT * KVD // 2
        nc.scalar.dma_start(wk_sb[:, 0:HK], wk[:, 0:HK])
        nc.gpsimd.dma_start(wv_sb[:, 0:HK], wv[:, 0:HK])

        xkp = [xk_pool.tile([128, 2 * S], BF16, tag=f"xkp{g}", name=f"xkp{g}")
               for g in range(NKT // 2)]
        # fp8 x copy for Q-proj (needed from qproj(0) right after pass 1):
        # cast on-device from the bf16 chunks on the otherwise-idle DVE
        xkp8 = [xk_pool.tile([128, 2, S], F8E4, tag=f"xkp8_{g}",
                             name=f"xkp8_{g}") for g in range(NKT // 2)]
        for g in range(NKT // 2):
            eng = nc.sync if g % 2 == 0 else nc.gpsimd
            if g == 0:
                # halves: the first pass-1 matmul only needs kt=0
                eng.dma_start(xkp[0][:, 0:S], xT[:, 0:S])
                eng.dma_start(xkp[0][:, S:2 * S], xT[:, S:2 * S])
            else:
                eng.dma_start(xkp[g][:], xT[:, 2 * g * S:(2 * g + 2) * S])
        for g in range(NKT // 2):
            nc.vector.tensor_copy(xkp8[g][:], xkp[g][:])

        # keep sync/gpsimd dedicated to x during pass 1; everything else
        # (needed from pass 2 onward) streams on the scalar queue in
        # first-use order
        wq_sb = {}
        for h in range(4):
            wq_sb[h] = wq_pool.tile([128, NKT, D], F8E4, tag="wqh",
                                    name=f"wqh{h}")
        nc.scalar.dma_start(wk_sb[:, HK:], wk[:, HK:])
        nc.scalar.dma_start(wv_sb[:, HK:], wv[:, HK:])
        nc.scalar.dma_start(wq_sb[0][:], wq8[:, 0:NKT, :])
        tbl_sb = const.tile([128, 2304], BF16)
        nc.scalar.dma_start(tbl_sb[:], tbl[:])
        nc.scalar.dma_start(wq_sb[1][:], wq8[:, NKT:2 * NKT, :])
        nc.scalar.dma_start(wq_sb[2][:], wq8[:, 2 * NKT:3 * NKT, :])
        nc.scalar.dma_start(wq_sb[3][:], wq8[:, 3 * NKT:4 * NKT, :])

        def xkc(kt, a, b):
            g, r = kt // 2, kt % 2
            return xkp[g][:, r * S + a:r * S + b]

        cosT_sb = tbl_sb[:, 0:S]
        sinTs_sb = tbl_sb[:, S:2 * S]
        p64_sb = tbl_sb[:, 2 * S:2 * S + D]
        dmask_sb = tbl_sb[:, 2 * S + D:2 * S + 2 * D]

        ones_bf = const.tile([128, 1], BF16)
        nc.vector.memset(ones_bf[:], 1.0)

        ctxT = [ctxT_pool.tile([D, S], BF16, tag=f"ctxT{h}", name=f"ctxT{h}")
                for h in range(HPC)]
        kT = [kT_pool.tile([D, S], BF16, tag=f"kT{c}", name=f"kT{c}")
              for c in range(KVPC)]
        v_sb = v_pool.tile([128, NST * KVD], BF16)

        # ---- pass 1: stream kt chunks once -> all of K proj + V st0-3 ----
        with ExitStack() as pro:
            psP = pro.enter_context(tc.tile_pool(name="psP", bufs=1,
                                                 space="PSUM"))
            kps = {}
            for c in range(KVPC):
                for sc in range(2):
                    kps[(c, sc)] = psP.tile([128, 512], F32, tag=f"kp{c}{sc}",
                                            name=f"kp{c}{sc}")
            vps = [psP.tile([128, 512], F32, tag=f"vp{st}", name=f"vp{st}")
                   for st in range(4)]
            for kt in range(NKT):
                for c in range(KVPC):
                    for sc in range(2):
                        nc.tensor.matmul(
                            kps[(c, sc)][:],
                            wk_sb[:, kt * KVD + c * D:kt * KVD + (c + 1) * D],
                            xkc(kt, sc * 512, (sc + 1) * 512),
                            start=(kt == 0), stop=(kt == NKT - 1))
                for st in range(4):
                    nc.tensor.matmul(
                        vps[st][:, 0:KVD], xkc(kt, st * 128, (st + 1) * 128),
                        wv_sb[:, kt * KVD:(kt + 1) * KVD],
                        start=(kt == 0), stop=(kt == NKT - 1))
            # evacuations split across vector+scalar so the psP-pool
            # release barrier (gating pass-2's first matmuls) clears ~2x
            # sooner
            for c in range(KVPC):
                for sc in range(2):
                    nc.vector.tensor_copy(kT[c][:, sc * 512:(sc + 1) * 512],
                                          kps[(c, sc)][:])
            for st in range(4):
                nc.scalar.copy(v_sb[:, st * KVD:(st + 1) * KVD],
                               vps[st][:, 0:KVD])

        # ---- pass 2 + head loop PSUM pools: exactly 8 banks --------------
        with ExitStack() as mid:
            ps_sc = mid.enter_context(tc.tile_pool(name="ps_sc", bufs=2,
                                                   space="PSUM"))
            ps_q = mid.enter_context(tc.tile_pool(name="ps_q", bufs=2,
                                                  space="PSUM"))
            ps_dn = mid.enter_context(tc.tile_pool(name="ps_dn", bufs=1,
                                                   space="PSUM"))

            def rope_inplace(dst, rope_id):
                """RoPE in T orientation on a [128, S] bf16 tile in place.
                Partition rotate-by-64 via a P64 perm matmul into the ps_dn
                bank pair (idle between denominator uses; sign of the
                rotate is folded into sinTs), then 3 DVE ops."""
                rot = ps_dn.tile([128, 1024], F32, tag="dn",
                                 name=f"rot{rope_id}")
                for sc in range(2):
                    nc.tensor.matmul(rot[:, sc * 512:(sc + 1) * 512], p64_sb,
                                     dst[:, sc * 512:(sc + 1) * 512],
                                     start=True, stop=True)
                tmp = tmp_pool.tile([128, S], BF16, tag="ropetmp",
                                    name=f"rtmp{rope_id}")
                nc.vector.tensor_mul(tmp[:], rot[:], sinTs_sb)
                nc.vector.tensor_mul(dst[:], dst[:], cosT_sb)
                nc.vector.tensor_add(dst[:], dst[:], tmp[:])

            # pass 2: K rope -> fp8 Q proj heads 0/1 -> V st4-7, so the
            # qT copy + rope latency hides under the V matmuls. K ropes get
            # dedicated rot tiles + the scalar DMA queue (free here) so no
            # ring WAR chains them to the qT ropes.
            for c in range(KVPC):
                rope_inplace(kT[c], f"k{c}")

            qT = {}

            def qproj_psum(hq, sc, p0, p1, ps):
                # fp8 DoubleRow: each matmul contracts a PAIR of k-tiles
                # (2*128 rows) at 2 MACs/cell/cycle
                for p in range(p0, p1):
                    nc.tensor.matmul(
                        ps[:], wq_sb[hq][:, 2 * p:2 * p + 2, :],
                        xkp8[p][:, :, sc * 512:(sc + 1) * 512],
                        start=(p == 0), stop=(p == NKT // 2 - 1),
                        perf_mode=mybir.MatmulPerfMode.DoubleRow)

            for hq in range(2):
                qT[hq] = qT_pool.tile([D, S], BF16, tag="qTh",
                                      name=f"qT{hq}")
                for sc in range(2):
                    ps = ps_q.tile([128, 512], F32, tag="ps_q",
                                   name=f"q{hq}s{sc}")
                    qproj_psum(hq, sc, 0, NKT // 2, ps)
                    nc.scalar.copy(qT[hq][:, sc * 512:(sc + 1) * 512], ps[:])
                rope_inplace(qT[hq], f"q{hq}")


            # ---- head loop with 2-group scores lookahead: G0/G1 of head
            # h+1 are issued near the end of head h so its exp (the scalar
            # critical path) gets a head start --------------------------------
            eS_t = {}

            def scores_group(h, g):
                eS = eS_t[h]
                c = h // (HPC // KVPC)
                w = GB[g + 1] - GB[g]
                ps = ps_sc.tile([128, 1024], F32, tag="ps_sc",
                                name=f"sc{h}_{g}")
                for (kt, q0, q1, rel) in GROUP_PIECES[g]:
                    nc.tensor.matmul(
                        ps[:, rel:rel + q1 - q0],
                        kT[c][:, kt * 128:(kt + 1) * 128],
                        qT[h][:, q0:q1], start=True, stop=True)
                nc.scalar.activation(
                    eS[:, GB[g]:GB[g + 1]], ps[:, 0:w],
                    mybir.ActivationFunctionType.Exp, scale=SCALE / QSCALE)
                for kt in DIAG_G[g]:
                    off = ES_OFF[kt]
                    nc.vector.tensor_mul(eS[:, off:off + 128],
                                         eS[:, off:off + 128], dmask_sb)

            dnf_t = {}

            def begin_head(h):
                eS_t[h] = eS_pool.tile([128, ES_W], BF16, tag="eS",
                                       name=f"eS{h}")
                dnf_t[h] = tmp_pool.tile([128, 2048], BF16, tag="dnf",
                                         name=f"dnf{h}")
                scores_group(h, 0)
                scores_group(h, 1)

            begin_head(0)
            nc.vector.tensor_add(dnf_t[0][:, 0:384], eS_t[0][:, 128:512],
                                 eS_t[0][:, 1024:1408])
            nc.vector.tensor_add(dnf_t[0][:, 512:1024], eS_t[0][:, 512:1024],
                                 eS_t[0][:, 1408:1920])
            for st in range(4, NST):
                ps = ps_q.tile([128, 512], F32, tag="ps_q", name=f"vq{st}")
                for kt in range(NKT):
                    nc.tensor.matmul(
                        ps[:, 0:KVD], xkc(kt, st * 128, (st + 1) * 128),
                        wv_sb[:, kt * KVD:(kt + 1) * KVD],
                        start=(kt == 0), stop=(kt == NKT - 1))
                nc.vector.tensor_copy(v_sb[:, st * KVD:(st + 1) * KVD],
                                      ps[:, 0:KVD])
            wo_sb = None
            for h in range(HPC):
                c = h // (HPC // KVPC)  # local kv head
                hq = h + 2              # head whose Q-proj we compute now
                eS = eS_t[h]

                if hq < HPC:
                    qT[hq] = qT_pool.tile([D, S], BF16, tag="qTh",
                                          name=f"qT{hq}")
                    psq0 = ps_q.tile([128, 512], F32, tag="ps_q",
                                     name=f"q{hq}s0")
                dn = ps_dn.tile([128, 1024], F32, tag="dn", name=f"dn{h}")

                def op_partial(k):
                    # heads 6/7: partial O-proj (heads 0-5) in the PE slots
                    # the Q-proj pipeline no longer needs
                    st, ec = PRE_TILES[(h - 6) * 5 + k]
                    po2 = ps_q.tile([128, 512], F32, tag="ps_q",
                                    name=f"op{st}_{ec}")
                    for hh in range(6):
                        nc.tensor.matmul(
                            po2[:], ctxT[hh][:, st * 128:(st + 1) * 128],
                            wo_sb[:, hh * HID + ec * 512:
                                  hh * HID + (ec + 1) * 512],
                            start=(hh == 0), stop=(hh == 5))
                    ot2 = out_pool.tile([128, 512], F32, tag="ot",
                                        name=f"ot2_{st}_{ec}")
                    dst = out2[st * 128:(st + 1) * 128,
                               ec * 512:(ec + 1) * 512]
                    nc.vector.tensor_copy(ot2[:], po2[:])
                    if k % 2:
                        nc.gpsimd.dma_start(dst, ot2[:])
                    else:
                        nc.sync.dma_start(dst, ot2[:])

                dnf = dnf_t[h]

                def dn_mm(q0, q1, src, st_, sp_):
                    # denominator partial sums on the PE: ones-matmul over
                    # raw eS pieces or DVE-prefolded kt-pairs (dnf), PSUM-
                    # accumulated into dn[0, q0:q1] (~halves the PE columns)
                    nc.tensor.matmul(dn[:1, q0:q1], ones_bf[:], src,
                                     start=st_, stop=sp_)

                scores_group(h, 2)
                if hq < HPC:
                    qproj_psum(hq, 0, 0, 4, psq0)
                nc.vector.tensor_add(dnf[:, 384:512], eS[:, 2048:2176],
                                     eS[:, 2688:2816])
                dn_mm(0, 128, eS[:, 0:128], True, False)
                dn_mm(128, 512, dnf[:, 0:384], True, False)
                dn_mm(512, 1024, dnf[:, 512:1024], True, False)
                if hq >= HPC:
                    op_partial(0)
                scores_group(h, 3)
                if hq < HPC:
                    qproj_psum(hq, 0, 4, 6, psq0)
                else:
                    op_partial(1)
                nc.vector.tensor_add(dnf[:, 1024:1536], eS[:, 2176:2688],
                                     eS[:, 2816:3328])
                dn_mm(256, 384, eS[:, 1920:2048], False, False)
                dn_mm(384, 512, dnf[:, 384:512], False, True)
                scores_group(h, 4)
                if hq < HPC:
                    qproj_psum(hq, 0, 6, 8, psq0)
                    psq1 = ps_q.tile([128, 512], F32, tag="ps_q",
                                     name=f"q{hq}s1")
                    qproj_psum(hq, 1, 0, 4, psq1)
                else:
                    op_partial(2)
                if h + 1 < HPC:
                    # lookahead scores G0/G1 of head h+1 issued here so the
                    # exps complete well before head h+1's first score mms
                    begin_head(h + 1)
                nc.vector.tensor_add(dnf[:, 1536:1920], eS[:, 3456:3840],
                                     eS[:, 3840:4224])
                nc.vector.tensor_add(dnf[:, 1920:2048], eS[:, 4352:4480],
                                     eS[:, 4480:4608])
                dn_mm(512, 1024, dnf[:, 1024:1536], False, False)
                dn_mm(512, 640, eS[:, 3328:3456], False, False)
                dn_mm(640, 1024, dnf[:, 1536:1920], False, False)
                dn_mm(768, 896, eS[:, 4224:4352], False, False)
                dn_mm(896, 1024, dnf[:, 1920:2048], False, True)
                if hq < HPC:
                    qproj_psum(hq, 1, 4, 8, psq1)
                else:
                    op_partial(3)
                if hq < HPC:
                    # qT copies issued after the lookahead exps so the
                    # scalar exp stream is never interrupted mid-head
                    nc.scalar.copy(qT[hq][:, 0:512], psq0[:])
                    nc.scalar.copy(qT[hq][:, 512:1024], psq1[:])

                rc = rc_pool.tile([1, S], F32, tag="rc", name=f"rc{h}")
                nc.vector.reciprocal_approx_fast(rc[:1, 0:512],
                                                 dn[:1, 0:512])
                rb0 = tmp_pool.tile([128, 512], F32, tag="rbtmp",
                                    name=f"rb0_{h}")
                nc.gpsimd.partition_broadcast(rb0[:], rc[:1, 0:512])
                nc.vector.reciprocal_approx_fast(rc[:1, 512:1024],
                                                 dn[:1, 512:1024])
                rb1 = tmp_pool.tile([128, 512], F32, tag="rbtmp",
                                    name=f"rb1_{h}")
                nc.gpsimd.partition_broadcast(rb1[:], rc[:1, 512:1024])

                if hq < HPC:
                    rope_inplace(qT[hq], f"q{hq}")
                else:
                    op_partial(4)

                # ctx matmuls (ragged accumulate) into one [128,1024] tile
                # in the ps_dn ring (dn -> rot -> pcx order per head)
                pcx = ps_dn.tile([128, 1024], F32, tag="dn",
                                 name=f"pc_{h}")
                for j in range(2):
                    kts = [kt for kt in range(NST)
                           if max(128 * kt, j * 512) < (j + 1) * 512]
                    for kt in kts:
                        qlo = 128 * kt
                        lo = max(qlo, j * 512)
                        hi = (j + 1) * 512
                        nc.tensor.matmul(
                            pcx[:, lo:hi],
                            v_sb[:, kt * KVD + c * D:kt * KVD + (c + 1) * D],
                            eS[:, ES_OFF[kt] + lo - qlo:ES_OFF[kt] + hi - qlo],
                            start=(kt == kts[0]), stop=(kt == kts[-1]))
                nc.vector.tensor_mul(ctxT[h][:, 0:512], pcx[:, 0:512],
                                     rb0[:])
                nc.vector.tensor_mul(ctxT[h][:, 512:1024], pcx[:, 512:1024],
                                     rb1[:])

                if h + 1 < HPC:
                    eS1, dnf1 = eS_t[h + 1], dnf_t[h + 1]
                    nc.vector.tensor_add(dnf1[:, 0:384], eS1[:, 128:512],
                                         eS1[:, 1024:1408])
                    nc.vector.tensor_add(dnf1[:, 512:1024],
                                         eS1[:, 512:1024],
                                         eS1[:, 1408:1920])

                # staged weight prefetches
                if h < 4 and h + 4 < HPC:
                    wq_sb[h + 4] = wq_pool.tile([128, NKT, D], F8E4,
                                                tag="wqh", name=f"wqh{h+4}")
                    nc.gpsimd.dma_start(
                        wq_sb[h + 4][:],
                        wq8[:, (h + 4) * NKT:(h + 5) * NKT, :])
                if h == 2:  # wo arrives while attention still running
                    wo_sb = wo_pool.tile([128, HPC * HID], BF16)
                    HW2 = HPC * HID // 2
                    nc.sync.dma_start(wo_sb[:, 0:HW2], wo[:, 0:HW2])
                    nc.gpsimd.dma_start(wo_sb[:, HW2:], wo[:, HW2:])

        # ---- O projection: own 8-bank PSUM pool, deep pipeline -----------
        with ExitStack() as fin:
            psO = fin.enter_context(tc.tile_pool(name="psO", bufs=8,
                                                 space="PSUM"))
            pre = set(PRE_TILES)
            for st in range(NST):
                for ec in range(HID // 512):
                    po = psO.tile([128, 512], F32, tag="po",
                                  name=f"po{st}_{ec}")
                    h0 = 6 if (st, ec) in pre else 0
                    for h in range(h0, HPC):
                        nc.tensor.matmul(
                            po[:], ctxT[h][:, st * 128:(st + 1) * 128],
                            wo_sb[:, h * HID + ec * 512:h * HID + (ec + 1) * 512],
                            start=(h == h0), stop=(h == HPC - 1))
                    ot = out_pool.tile([128, 512], F32, tag="ot")
                    if (st * 4 + ec) % 2:
                        nc.scalar.copy(ot[:], po[:])
                        nc.gpsimd.dma_start(
                            out[st * 128:(st + 1) * 128,
                                ec * 512:(ec + 1) * 512], ot[:])
                    else:
                        nc.vector.tensor_copy(ot[:], po[:])
                        nc.sync.dma_start(
                            out[st * 128:(st + 1) * 128,
                                ec * 512:(ec + 1) * 512], ot[:])
    nc.finalize()
    return nc


def host_prep(hidden_states, Wq, Wk, Wv, Wo):
    """Pre-transpose/cast/relayout all inputs on the host (bf16 + fp8)."""
    bf = ml_dtypes.bfloat16
    f8 = ml_dtypes.float8_e4m3fn
    xTs = []
    for b in range(B):
        t = hidden_states[b].T.reshape(NKT, 128, S).transpose(1, 0, 2)
        xTs.append(np.ascontiguousarray(t.astype(bf)).reshape(128, NKT * S))
    halves = []
    for hf in range(2):
        wqh = Wq[:, 1024 * hf:1024 * (hf + 1)].reshape(NKT, 128, HPC, D)
        wqh = np.ascontiguousarray(
            np.clip(wqh.transpose(1, 2, 0, 3) * QSCALE, -240, 240)
            .astype(f8)).reshape(128, HPC * NKT * D)
        wkh = Wk[:, KVD * hf:KVD * (hf + 1)].reshape(NKT, 128, KVD)
        wkh = np.ascontiguousarray(
            wkh.transpose(1, 0, 2).astype(bf)).reshape(128, NKT * KVD)
        wvh = Wv[:, KVD * hf:KVD * (hf + 1)].reshape(NKT, 128, KVD)
        wvh = np.ascontiguousarray(
            wvh.transpose(1, 0, 2).astype(bf)).reshape(128, NKT * KVD)
        woh = Wo[1024 * hf:1024 * (hf + 1), :].reshape(HPC, 128, HID)
        woh = np.ascontiguousarray(
            woh.transpose(1, 0, 2).astype(bf)).reshape(128, HPC * HID)
        halves.append((wqh, wkh, wvh, woh))

    inv_freq = 1.0 / (10000.0 ** (np.arange(0, D, 2, dtype=np.float64) / D))
    t = np.arange(S, dtype=np.float64)
    freqs = np.outer(t, inv_freq)
    emb = np.concatenate([freqs, freqs], -1)
    cosT = np.cos(emb).T
    sinTs_f = np.sin(emb).T.copy()
    sinTs_f[:64] *= -1.0
    p64 = np.zeros((D, D), dtype=np.float64)
    for d in range(D):
        p64[d, (d + 64) % D] = 1.0
    dmask = np.triu(np.ones((128, 128), dtype=np.float64))
    tbl = np.ascontiguousarray(
        np.concatenate([cosT, sinTs_f, p64, dmask], axis=1)).astype(bf)
    return xTs, halves, tbl


_CACHE = {}


def kernel(hidden_states, Wq, Wk, Wv, Wo, _trace=False, _tmpdir=None):
    hidden_states = np.ascontiguousarray(hidden_states, dtype=np.float32)
    Wq = np.ascontiguousarray(Wq, dtype=np.float32)
    Wk = np.ascontiguousarray(Wk, dtype=np.float32)
    Wv = np.ascontiguousarray(Wv, dtype=np.float32)
    Wo = np.ascontiguousarray(Wo, dtype=np.float32)

    if "nc" not in _CACHE:
        _CACHE["nc"] = build_kernel()
    nc = _CACHE["nc"]
    xTs, halves, tbl = host_prep(hidden_states, Wq, Wk, Wv, Wo)

    in_maps = []
    for cid in range(8):
        b, hf = cid // 2, cid % 2
        wqh, wkh, wvh, woh = halves[hf]
        in_maps.append({
            "xT": xTs[b], "wq8": wqh, "wk": wkh, "wv": wvh,
            "wo": woh, "tbl": tbl,
        })
    res = run_bass_kernel_spmd(nc, in_maps, list(range(8)),
                               trace=_trace, tmpdir=_tmpdir)
    out = np.zeros((B, S, HID), dtype=np.float32)
    for cid in range(8):
        out[cid // 2] += res.results[cid]["out"]
        out[cid // 2][0:640, 0:1024] += res.results[cid]["out2"]
    if _trace:
        return out, res
    return out


# revision 54
# speedup vs baseline: 1.2078x; 1.0152x over previous
"""GQA attention kernel for Trainium2, sharded over 8 NeuronCores.

Problem (hardcoded): B=4, S=1024, HID=2048, 16 query heads, 4 KV heads,
head_dim=128, RoPE (base 10000), causal softmax, O-projection.

Sharding: core c handles (batch b = c//2, head-half = c%2): 8 query heads,
2 KV heads, and the matching column/row shards of Wq/Wk/Wv/Wo. Each core
produces a partial O-projection output [S, HID] (plus a small partial-O
side output); the host sums the per-batch partials.

v4 design (evolved from the 286us v2 baseline via perfetto analysis;
measures ~200-204us warm / ~235us when the chip P0-downclocks):
- scores->exp in 5 ragged PSUM groups per head: one [128,<=1024] two-bank
  scalar ACTIVATE per group instead of 12 narrow ones (the 352-cycle/instr
  ACT overhead was half the scalar time, and PSUM recycling had been
  gating the score matmuls at scalar speed).
- softmax denominator accumulated ON the PE: ones-matmuls into a
  dedicated [1,1024] PSUM bank pair, interleaved right after each exp
  group; the idle Vector engine pre-folds kt-pairs (6 bf16 adds) so the
  PE only contracts 2560 of the 4608 columns. This replaced the v2/v3
  serial pre-sum chains whose cross-engine latency caused 5-9us PE
  stalls per head and HAM re-throttling to 1.2GHz.
- Q-projection in fp8 e4m3 with DoubleRow (2 k-tiles per matmul, 2
  MACs/cell/cycle): Wq is host-prescaled by 64 (folded into the exp
  scale); the fp8 x copy is cast on-device from the bf16 stream by the
  idle DVE during pass 1 (saves 2MB of the DMA-bound prologue). Softmax
  compresses the quantization error: rel err 0.0123 vs the 0.02 gate
  (bf16 was 0.0026).
  K/V/O stay bf16 - their error hits the output linearly and the max-err
  tails do not fit the budget.
- software pipeline: Q-proj of head h+2 and score groups G0/G1 of head
  h+1 are interleaved into head h, so the exp chain (the scalar-engine
  critical path) starts ~4us early and the PE never waits on it.
- RoPE via a P64 permutation matmul into the ps_dn bank pair + 3 DVE ops
  (sign folded into the sin table). An SBUF->SBUF DMA rotate variant was
  faster on paper but kept serializing behind Tile's counting-semaphore
  aliasing across DMA queues.
- partial O-projection: heads 6/7 have no Q-proj left to pipeline, so 10
  output tiles accumulate their heads-0-5 contribution there, shipped via
  a second DRAM output that the host adds in; the final O-proj phase then
  only needs 2 matmuls for those tiles (-15us of serial tail).
- PSUM budget (8 banks): ps_sc [128,1024]x2 holds ONLY the five score
  groups (>=2.5 groups of slack vs their exps), ps_q [128,512]x2 for
  Q-proj/partial-O, ps_dn [128,1024]x1 cycles denominator -> rope
  scratch -> ctx within each head. Ring orders are chosen so no
  allocation waits on later-program-order work.
- DMA: 3 queues; first pass-1 matmul needs only the first transfer of
  each queue; x (bf16+fp8) streams ahead of pass-1 consumption; Wo and
  the tail Wq tiles prefetch mid-head-loop; output stores round-robin
  sync/gpsimd with an 8-bank O-proj PSUM ring.

Per-core layouts (partition dim first, 128 everywhere):
  xT   [128, kt(16), s(1024)]   xT[p,kt,s] = x[s, 128kt+p]          bf16
  xkp8 x re-cast to fp8 on-device, kt-pair-major for DoubleRow     fp8e4
  wq8  [128, h(8)*kt(16), d(128)]  64*Wq                            fp8e4
  wk/wv[128, kt(16), c(2)*d(128)]                                   bf16
  wo   [128, h(8), e(2048)]     wo[p,h,e] = Wo[1024*half+128h+p, e] bf16
  qT/kT[128d, s]   roped in T orientation
  v    [128s, st(8)*c(2)*d(128)]  natural, for ctx stationary
  eS   [128k, ragged q spans]   exp(scores^T) bf16, 4608 cols
  ctxT [128d, s] per head       normalized context, feeds O-proj
"""
import math
from contextlib import ExitStack

import numpy as np
import ml_dtypes

import concourse.bass as bass
import concourse.bacc as bacc
import concourse.tile as tile
from concourse import mybir
from concourse.bass_utils import run_bass_kernel_spmd

F32 = mybir.dt.float32
BF16 = mybir.dt.bfloat16
F8E4 = mybir.dt.float8e4
QSCALE = 64.0  # host premultiplies Wq by this before fp8; folded into exp

B, S, HID = 4, 1024, 2048
NH, NKV, D = 16, 4, 128
HPC = 8          # query heads per core
KVPC = 2         # kv heads per core
SCALE = 1.0 / math.sqrt(D)
NKT = HID // 128  # 16 contraction tiles
NST = S // 128    # 8 sequence tiles
KVD = KVPC * D    # 256

# eS ragged packing: block for key-tile kt covers q in [128kt, S), stored at
# eS col ES_OFF[kt] + (q - 128kt).
ES_OFF = []
_o = 0
for _kt in range(NST):
    ES_OFF.append(_o)
    _o += S - 128 * _kt
ES_W = _o  # 4608

# exp groups: eS col ranges, each <=1024 wide (one [128,1024] 2-bank PSUM
# tile + one wide ACTIVATE). Pieces never cross a 512 (bank) boundary.
GB = [0, 1024, 2048, 3072, 4096, ES_W]
NG = 5


def _group_pieces(g):
    glo, ghi = GB[g], GB[g + 1]
    out = []
    for kt in range(NST):
        blo = ES_OFF[kt]
        bhi = blo + (S - 128 * kt)
        lo, hi = max(blo, glo), min(bhi, ghi)
        if lo >= hi:
            continue
        p = lo
        while p < hi:
            nxt = min(hi, (p // 512 + 1) * 512)
            out.append((kt, 128 * kt + (p - blo), 128 * kt + (nxt - blo),
                        p - glo))
            p = nxt
    return out


GROUP_PIECES = [_group_pieces(g) for g in range(NG)]
# O-proj tiles whose heads 0-5 partial is precomputed during heads 6/7
PRE_TILES = [(st, ec) for st in range(5) for ec in range(2)]
# which kts have their (diagonal-masked) block start inside group g
DIAG_G = [[kt for kt in range(NST) if GB[g] <= ES_OFF[kt] < GB[g + 1]]
          for g in range(NG)]

# denominator accumulation pieces: per kt, the q-span [128kt, S) split at the
# j boundary (q=512, the dn PSUM bank edge). Each piece is issued after the
# exp group that contains its last eS column (groups complete in order).
# kt0 pieces carry start=True (kt0 spans all q, so it initializes both
# banks); the final writer of each bank carries stop=True.
DN_G = [[] for _ in range(NG)]
for _kt in range(NST):
    _spans = [(128 * _kt, 512), (512, S)] if 128 * _kt < 512 else \
        [(128 * _kt, S)]
    for (_q0, _q1) in _spans:
        _lastcol = ES_OFF[_kt] + (_q1 - 1) - 128 * _kt
        _g = next(g for g in range(NG) if _lastcol < GB[g + 1])
        _stop = (_kt == 3 and _q1 == 512) or (_kt == 7)
        DN_G[_g].append((_kt, _q0, _q1, _kt == 0, _stop))


def build_kernel():
    nc = bacc.Bacc(None)
    xT = nc.dram_tensor("xT", [128, NKT * S], BF16, kind="ExternalInput")
    # fp8 Wq for the Q projection (DoubleRow: 2 k-tiles per pass); the fp8
    # x copy is cast on-device from the bf16 stream (saves 2MB of the
    # DMA-bound prologue)
    wq8 = nc.dram_tensor("wq8", [128, HPC * NKT, D], F8E4,
                         kind="ExternalInput")
    wk = nc.dram_tensor("wk", [128, NKT * KVD], BF16, kind="ExternalInput")
    wv = nc.dram_tensor("wv", [128, NKT * KVD], BF16, kind="ExternalInput")
    wo = nc.dram_tensor("wo", [128, HPC * HID], BF16, kind="ExternalInput")
    # tables bundled: cosT(1024) | sinTs(1024) | p64(128, unused) | dmask(128)
    tbl = nc.dram_tensor("tbl", [128, 2304], BF16, kind="ExternalInput")
    out = nc.dram_tensor("out", [S, HID], F32, kind="ExternalOutput")
    # partial O-proj (heads 0-5) for 10 tiles, computed during heads 6/7 in
    # the PE slots freed by the ended Q-proj pipeline; host adds this into
    # out[0:640, 0:1024]
    out2 = nc.dram_tensor("out2", [5 * 128, 2 * 512], F32,
                          kind="ExternalOutput")

    with tile.TileContext(nc) as tc, ExitStack() as top:
        const = top.enter_context(tc.tile_pool(name="const", bufs=1))
        xk_pool = top.enter_context(tc.tile_pool(name="xk", bufs=1))
        wkv_pool = top.enter_context(tc.tile_pool(name="wkv", bufs=1))
        kT_pool = top.enter_context(tc.tile_pool(name="kT", bufs=1))
        v_pool = top.enter_context(tc.tile_pool(name="v", bufs=1))
        ctxT_pool = top.enter_context(tc.tile_pool(name="ctxT", bufs=1))
        wq_pool = top.enter_context(tc.tile_pool(name="wq", bufs=4))
        qT_pool = top.enter_context(tc.tile_pool(name="qT", bufs=3))
        eS_pool = top.enter_context(tc.tile_pool(name="eS", bufs=2))
        rc_pool = top.enter_context(tc.tile_pool(name="rc", bufs=2))
        tmp_pool = top.enter_context(tc.tile_pool(name="tmp", bufs=2))
        wo_pool = top.enter_context(tc.tile_pool(name="wo", bufs=1))
        out_pool = top.enter_context(tc.tile_pool(name="outp", bufs=6))

        # ---- input DMA over the 3 DMA-capable queues (sync/gpsimd/scalar);
        # first pass-1 matmul needs only wk_h0 (scalar q), wv_h0 (gpsimd q),
        # xkp0 (sync q) — each is the first transfer on its queue ----------
        wk_sb = wkv_pool.tile([128, NKT * KVD], BF16, tag="wk")
        wv_sb = wkv_pool.tile([128, NKT * KVD], BF16, tag="wv")
        HK = NKT * KVD // 2
        nc.scalar.dma_start(wk_sb[:, 0:HK], wk[:, 0:HK])
        nc.gpsimd.dma_start(wv_sb[:, 0:HK], wv[:, 0:HK])

        xkp = [xk_pool.tile([128, 2 * S], BF16, tag=f"xkp{g}", name=f"xkp{g}")
               for g in range(NKT // 2)]
        # fp8 x copy for Q-proj (needed from qproj(0) right after pass 1):
        # cast on-device from the bf16 chunks on the otherwise-idle DVE
        xkp8 = [xk_pool.tile([128, 2, S], F8E4, tag=f"xkp8_{g}",
                             name=f"xkp8_{g}") for g in range(NKT // 2)]
        for g in range(NKT // 2):
            eng = nc.sync if g % 2 == 0 else nc.gpsimd
            if g == 0:
                # halves: the first pass-1 matmul only needs kt=0
                eng.dma_start(xkp[0][:, 0:S], xT[:, 0:S])
                eng.dma_start(xkp[0][:, S:2 * S], xT[:, S:2 * S])
            else:
                eng.dma_start(xkp[g][:], xT[:, 2 * g * S:(2 * g + 2) * S])
        for g in range(NKT // 2):
            nc.vector.tensor_copy(xkp8[g][:], xkp[g][:])

        # keep sync/gpsimd dedicated to x during pass 1; everything else
        # (needed from pass 2 onward) streams on the scalar queue in
        # first-use order
        wq_sb = {}
        for h in range(4):
            wq_sb[h] = wq_pool.tile([128, NKT, D], F8E4, tag="wqh",
                                    name=f"wqh{h}")
        nc.scalar.dma_start(wk_sb[:, HK:], wk[:, HK:])
        nc.scalar.dma_start(wv_sb[:, HK:], wv[:, HK:])
        nc.scalar.dma_start(wq_sb[0][:], wq8[:, 0:NKT, :])
        tbl_sb = const.tile([128, 2304], BF16)
        nc.scalar.dma_start(tbl_sb[:], tbl[:])
        nc.scalar.dma_start(wq_sb[1][:], wq8[:, NKT:2 * NKT, :])
        nc.scalar.dma_start(wq_sb[2][:], wq8[:, 2 * NKT:3 * NKT, :])
        nc.scalar.dma_start(wq_sb[3][:], wq8[:, 3 * NKT:4 * NKT, :])

        def xkc(kt, a, b):
            g, r = kt // 2, kt % 2
            return xkp[g][:, r * S + a:r * S + b]

        cosT_sb = tbl_sb[:, 0:S]
        sinTs_sb = tbl_sb[:, S:2 * S]
        p64_sb = tbl_sb[:, 2 * S:2 * S + D]
        dmask_sb = tbl_sb[:, 2 * S + D:2 * S + 2 * D]

        ones_bf = const.tile([128, 1], BF16)
        nc.vector.memset(ones_bf[:], 1.0)

        ctxT = [ctxT_pool.tile([D, S], BF16, tag=f"ctxT{h}", name=f"ctxT{h}")
                for h in range(HPC)]
        kT = [kT_pool.tile([D, S], BF16, tag=f"kT{c}", name=f"kT{c}")
              for c in range(KVPC)]
        v_sb = v_pool.tile([128, NST * KVD], BF16)

        # ---- pass 1: stream kt chunks once -> all of K proj + V st0-3 ----
        with ExitStack() as pro:
            psP = pro.enter_context(tc.tile_pool(name="psP", bufs=1,
                                                 space="PSUM"))
            kps = {}
            for c in range(KVPC):
                for sc in range(2):
                    kps[(c, sc)] = psP.tile([128, 512], F32, tag=f"kp{c}{sc}",
                                            name=f"kp{c}{sc}")
            vps = [psP.tile([128, 512], F32, tag=f"vp{st}", name=f"vp{st}")
                   for st in range(4)]
            for kt in range(NKT):
                for c in range(KVPC):
                    for sc in range(2):
                        nc.tensor.matmul(
                            kps[(c, sc)][:],
                            wk_sb[:, kt * KVD + c * D:kt * KVD + (c + 1) * D],
                            xkc(kt, sc * 512, (sc + 1) * 512),
                            start=(kt == 0), stop=(kt == NKT - 1))
                for st in range(4):
                    nc.tensor.matmul(
                        vps[st][:, 0:KVD], xkc(kt, st * 128, (st + 1) * 128),
                        wv_sb[:, kt * KVD:(kt + 1) * KVD],
                        start=(kt == 0), stop=(kt == NKT - 1))
            # evacuations split across vector+scalar so the psP-pool
            # release barrier (gating pass-2's first matmuls) clears ~2x
            # sooner
            for c in range(KVPC):
                for sc in range(2):
                    nc.vector.tensor_copy(kT[c][:, sc * 512:(sc + 1) * 512],
                                          kps[(c, sc)][:])
            for st in range(4):
                nc.scalar.copy(v_sb[:, st * KVD:(st + 1) * KVD],
                               vps[st][:, 0:KVD])

        # ---- pass 2 + head loop PSUM pools: exactly 8 banks --------------
        with ExitStack() as mid:
            ps_sc = mid.enter_context(tc.tile_pool(name="ps_sc", bufs=2,
                                                   space="PSUM"))
            ps_q = mid.enter_context(tc.tile_pool(name="ps_q", bufs=2,
                                                  space="PSUM"))
            ps_dn = mid.enter_context(tc.tile_pool(name="ps_dn", bufs=1,
                                                   space="PSUM"))

            def rope_inplace(dst, rope_id):
                """RoPE in T orientation on a [128, S] bf16 tile in place.
                Partition rotate-by-64 via a P64 perm matmul into the ps_dn
                bank pair (idle between denominator uses; sign of the
                rotate is folded into sinTs), then 3 DVE ops."""
                rot = ps_dn.tile([128, 1024], F32, tag="dn",
                                 name=f"rot{rope_id}")
                for sc in range(2):
                    nc.tensor.matmul(rot[:, sc * 512:(sc + 1) * 512], p64_sb,
                                     dst[:, sc * 512:(sc + 1) * 512],
                                     start=True, stop=True)
                tmp = tmp_pool.tile([128, S], BF16, tag="ropetmp",
                                    name=f"rtmp{rope_id}")
                nc.vector.tensor_mul(tmp[:], rot[:], sinTs_sb)
                nc.vector.tensor_mul(dst[:], dst[:], cosT_sb)
                nc.vector.tensor_add(dst[:], dst[:], tmp[:])

            # pass 2: K rope -> fp8 Q proj heads 0/1 -> V st4-7, so the
            # qT copy + rope latency hides under the V matmuls. K ropes get
            # dedicated rot tiles + the scalar DMA queue (free here) so no
            # ring WAR chains them to the qT ropes.
            for c in range(KVPC):
                rope_inplace(kT[c], f"k{c}")

            qT = {}

            def qproj_psum(hq, sc, p0, p1, ps):
                # fp8 DoubleRow: each matmul contracts a PAIR of k-tiles
                # (2*128 rows) at 2 MACs/cell/cycle
                for p in range(p0, p1):
                    nc.tensor.matmul(
                        ps[:], wq_sb[hq][:, 2 * p:2 * p + 2, :],
                        xkp8[p][:, :, sc * 512:(sc + 1) * 512],
                        start=(p == 0), stop=(p == NKT // 2 - 1),
                        perf_mode=mybir.MatmulPerfMode.DoubleRow)

            for hq in range(2):
                qT[hq] = qT_pool.tile([D, S], BF16, tag="qTh",
                                      name=f"qT{hq}")
                for sc in range(2):
                    ps = ps_q.tile([128, 512], F32, tag="ps_q",
                                   name=f"q{hq}s{sc}")
                    qproj_psum(hq, sc, 0, NKT // 2, ps)
                    nc.scalar.copy(qT[hq][:, sc * 512:(sc + 1) * 512], ps[:])
                rope_inplace(qT[hq], f"q{hq}")


            # ---- head loop with 2-group scores lookahead: G0/G1 of head
            # h+1 are issued near the end of head h so its exp (the scalar
            # critical path) gets a head start --------------------------------
            eS_t = {}

            def scores_group(h, g):
                eS = eS_t[h]
                c = h // (HPC // KVPC)
                w = GB[g + 1] - GB[g]
                ps = ps_sc.tile([128, 1024], F32, tag="ps_sc",
                                name=f"sc{h}_{g}")
                for (kt, q0, q1, rel) in GROUP_PIECES[g]:
                    nc.tensor.matmul(
                        ps[:, rel:rel + q1 - q0],
                        kT[c][:, kt * 128:(kt + 1) * 128],
                        qT[h][:, q0:q1], start=True, stop=True)
                nc.scalar.activation(
                    eS[:, GB[g]:GB[g + 1]], ps[:, 0:w],
                    mybir.ActivationFunctionType.Exp, scale=SCALE / QSCALE)
                for kt in DIAG_G[g]:
                    off = ES_OFF[kt]
                    nc.vector.tensor_mul(eS[:, off:off + 128],
                                         eS[:, off:off + 128], dmask_sb)

            dnf_t = {}

            def begin_head(h):
                eS_t[h] = eS_pool.tile([128, ES_W], BF16, tag="eS",
                                       name=f"eS{h}")
                dnf_t[h] = tmp_pool.tile([128, 2048], BF16, tag="dnf",
                                         name=f"dnf{h}")
                scores_group(h, 0)
                scores_group(h, 1)

            begin_head(0)
            nc.vector.tensor_add(dnf_t[0][:, 0:384], eS_t[0][:, 128:512],
                                 eS_t[0][:, 1024:1408])
            nc.vector.tensor_add(dnf_t[0][:, 512:1024], eS_t[0][:, 512:1024],
                                 eS_t[0][:, 1408:1920])
            for st in range(4, NST):
                ps = ps_q.tile([128, 512], F32, tag="ps_q", name=f"vq{st}")
                for kt in range(NKT):
                    nc.tensor.matmul(
                        ps[:, 0:KVD], xkc(kt, st * 128, (st + 1) * 128),
                        wv_sb[:, kt * KVD:(kt + 1) * KVD],
                        start=(kt == 0), stop=(kt == NKT - 1))
                nc.vector.tensor_copy(v_sb[:, st * KVD:(st + 1) * KVD],
                                      ps[:, 0:KVD])
            wo_sb = None
            for h in range(HPC):
                c = h // (HPC // KVPC)  # local kv head
                hq = h + 2              # head whose Q-proj we compute now
                eS = eS_t[h]

                if hq < HPC:
                    qT[hq] = qT_pool.tile([D, S], BF16, tag="qTh",
                                          name=f"qT{hq}")
                    psq0 = ps_q.tile([128, 512], F32, tag="ps_q",
                                     name=f"q{hq}s0")
                dn = ps_dn.tile([128, 1024], F32, tag="dn", name=f"dn{h}")

                def op_partial(k):
                    # heads 6/7: partial O-proj (heads 0-5) in the PE slots
                    # the Q-proj pipeline no longer needs
                    st, ec = PRE_TILES[(h - 6) * 5 + k]
                    po2 = ps_q.tile([128, 512], F32, tag="ps_q",
                                    name=f"op{st}_{ec}")
                    for hh in range(6):
                        nc.tensor.matmul(
                            po2[:], ctxT[hh][:, st * 128:(st + 1) * 128],
                            wo_sb[:, hh * HID + ec * 512:
                                  hh * HID + (ec + 1) * 512],
                            start=(hh == 0), stop=(hh == 5))
                    ot2 = out_pool.tile([128, 512], F32, tag="ot",
                                        name=f"ot2_{st}_{ec}")
                    dst = out2[st * 128:(st + 1) * 128,
                               ec * 512:(ec + 1) * 512]
                    nc.vector.tensor_copy(ot2[:], po2[:])
                    if k % 2:
                        nc.gpsimd.dma_start(dst, ot2[:])
                    else:
                        nc.sync.dma_start(dst, ot2[:])

                dnf = dnf_t[h]

                def dn_mm(q0, q1, src, st_, sp_):
                    # denominator partial sums on the PE: ones-matmul over
                    # raw eS pieces or DVE-prefolded kt-pairs (dnf), PSUM-
                    # accumulated into dn[0, q0:q1] (~halves the PE columns)
                    nc.tensor.matmul(dn[:1, q0:q1], ones_bf[:], src,
                                     start=st_, stop=sp_)

                scores_group(h, 2)
                if hq < HPC:
                    qproj_psum(hq, 0, 0, 4, psq0)
                nc.vector.tensor_add(dnf[:, 384:512], eS[:, 2048:2176],
                                     eS[:, 2688:2816])
                dn_mm(0, 128, eS[:, 0:128], True, False)
                dn_mm(128, 512, dnf[:, 0:384], True, False)
                dn_mm(512, 1024, dnf[:, 512:1024], True, False)
                if hq >= HPC:
                    op_partial(0)
                scores_group(h, 3)
                if hq < HPC:
                    qproj_psum(hq, 0, 4, 6, psq0)
                else:
                    op_partial(1)
                nc.vector.tensor_add(dnf[:, 1024:1536], eS[:, 2176:2688],
                                     eS[:, 2816:3328])
                dn_mm(256, 384, eS[:, 1920:2048], False, False)
                dn_mm(384, 512, dnf[:, 384:512], False, True)
                scores_group(h, 4)
                if hq < HPC:
                    qproj_psum(hq, 0, 6, 8, psq0)
                    psq1 = ps_q.tile([128, 512], F32, tag="ps_q",
                                     name=f"q{hq}s1")
                    qproj_psum(hq, 1, 0, 4, psq1)
                else:
                    op_partial(2)
                if h + 1 < HPC:
                    # lookahead scores G0/G1 of head h+1 issued here so the
                    # exps complete well before head h+1's first score mms
                    begin_head(h + 1)
                nc.vector.tensor_add(dnf[:, 1536:1920], eS[:, 3456:3840],
                                     eS[:, 3840:4224])
                nc.vector.tensor_add(dnf[:, 1920:2048], eS[:, 4352:4480],
                                     eS[:, 4480:4608])
                dn_mm(512, 1024, dnf[:, 1024:1536], False, False)
                dn_mm(512, 640, eS[:, 3328:3456], False, False)
                dn_mm(640, 1024, dnf[:, 1536:1920], False, False)
                dn_mm(768, 896, eS[:, 4224:4352], False, False)
                dn_mm(896, 1024, dnf[:, 1920:2048], False, True)
                if hq < HPC:
                    qproj_psum(hq, 1, 4, 8, psq1)
                else:
                    op_partial(3)
                if hq < HPC:
                    # qT copies issued after the lookahead exps so the
                    # scalar exp stream is never interrupted mid-head
                    nc.scalar.copy(qT[hq][:, 0:512], psq0[:])
                    nc.scalar.copy(qT[hq][:, 512:1024], psq1[:])

                rc = rc_pool.tile([1, S], F32, tag="rc", name=f"rc{h}")
                nc.vector.reciprocal_approx_fast(rc[:1, 0:512],
                                                 dn[:1, 0:512])
                rb0 = tmp_pool.tile([128, 512], F32, tag="rbtmp",
                                    name=f"rb0_{h}")
                nc.gpsimd.partition_broadcast(rb0[:], rc[:1, 0:512])
                nc.vector.reciprocal_approx_fast(rc[:1, 512:1024],
                                                 dn[:1, 512:1024])
                rb1 = tmp_pool.tile([128, 512], F32, tag="rbtmp",
                                    name=f"rb1_{h}")
                nc.gpsimd.partition_broadcast(rb1[:], rc[:1, 512:1024])

                if hq < HPC:
                    rope_inplace(qT[hq], f"q{hq}")
                else:
                    op_partial(4)

                # ctx matmuls (ragged accumulate) into one [128,1024] tile
                # in the ps_dn ring (dn -> rot -> pcx order per head)
                pcx = ps_dn.tile([128, 1024], F32, tag="dn",
                                 name=f"pc_{h}")
                for j in range(2):
                    kts = [kt for kt in range(NST)
                           if max(128 * kt, j * 512) < (j + 1) * 512]
                    for kt in kts:
                        qlo = 128 * kt
                        lo = max(qlo, j * 512)
                        hi = (j + 1) * 512
                        nc.tensor.matmul(
                            pcx[:, lo:hi],
                            v_sb[:, kt * KVD + c * D:kt * KVD + (c + 1) * D],
                            eS[:, ES_OFF[kt] + lo - qlo:ES_OFF[kt] + hi - qlo],
                            start=(kt == kts[0]), stop=(kt == kts[-1]))
                nc.vector.tensor_mul(ctxT[h][:, 0:512], pcx[:, 0:512],
                                     rb0[:])
                nc.vector.tensor_mul(ctxT[h][:, 512:1024], pcx[:, 512:1024],
                                     rb1[:])

                if h + 1 < HPC:
                    eS1, dnf1 = eS_t[h + 1], dnf_t[h + 1]
                    nc.vector.tensor_add(dnf1[:, 0:384], eS1[:, 128:512],
                                         eS1[:, 1024:1408])
                    nc.vector.tensor_add(dnf1[:, 512:1024],
                                         eS1[:, 512:1024],
                                         eS1[:, 1408:1920])

                # staged weight prefetches
                if h < 4 and h + 4 < HPC:
                    wq_sb[h + 4] = wq_pool.tile([128, NKT, D], F8E4,
                                                tag="wqh", name=f"wqh{h+4}")
                    nc.gpsimd.dma_start(
                        wq_sb[h + 4][:],
                        wq8[:, (h + 4) * NKT:(h + 5) * NKT, :])
                if h == 2:  # wo arrives while attention still running
                    wo_sb = wo_pool.tile([128, HPC * HID], BF16)
                    HW2 = HPC * HID // 2
                    nc.sync.dma_start(wo_sb[:, 0:HW2], wo[:, 0:HW2])
                    nc.gpsimd.dma_start(wo_sb[:, HW2:], wo[:, HW2:])

        # ---- O projection: own 8-bank PSUM pool, deep pipeline -----------
        with ExitStack() as fin:
            psO = fin.enter_context(tc.tile_pool(name="psO", bufs=8,
                                                 space="PSUM"))
            pre = set(PRE_TILES)
            for st in range(NST):
                for ec in range(HID // 512):
                    po = psO.tile([128, 512], F32, tag="po",
                                  name=f"po{st}_{ec}")
                    h0 = 6 if (st, ec) in pre else 0
                    for h in range(h0, HPC):
                        nc.tensor.matmul(
                            po[:], ctxT[h][:, st * 128:(st + 1) * 128],
                            wo_sb[:, h * HID + ec * 512:h * HID + (ec + 1) * 512],
                            start=(h == h0), stop=(h == HPC - 1))
                    ot = out_pool.tile([128, 512], F32, tag="ot")
                    if (st * 4 + ec) % 2:
                        nc.scalar.copy(ot[:], po[:])
                        nc.gpsimd.dma_start(
                            out[st * 128:(st + 1) * 128,
                                ec * 512:(ec + 1) * 512], ot[:])
                    else:
                        nc.vector.tensor_copy(ot[:], po[:])
                        nc.sync.dma_start(
                            out[st * 128:(st + 1) * 128,
                                ec * 512:(ec + 1) * 512], ot[:])
    nc.finalize()
    return nc


def host_prep(hidden_states, Wq, Wk, Wv, Wo):
    """Pre-transpose/cast/relayout all inputs on the host (bf16 + fp8)."""
    bf = ml_dtypes.bfloat16
    f8 = ml_dtypes.float8_e4m3fn
    xTs = []
    for b in range(B):
        t = hidden_states[b].T.reshape(NKT, 128, S).transpose(1, 0, 2)
        xTs.append(np.ascontiguousarray(t.astype(bf)).reshape(128, NKT * S))
    halves = []
    for hf in range(2):
        wqh = Wq[:, 1024 * hf:1024 * (hf + 1)].reshape(NKT, 128, HPC, D)
        wqh = np.ascontiguousarray(
            np.clip(wqh.transpose(1, 2, 0, 3) * QSCALE, -240, 240)
            .astype(f8)).reshape(128, HPC * NKT * D)
        wkh = Wk[:, KVD * hf:KVD * (hf + 1)].reshape(NKT, 128, KVD)
        wkh = np.ascontiguousarray(
            wkh.transpose(1, 0, 2).astype(bf)).reshape(128, NKT * KVD)
        wvh = Wv[:, KVD * hf:KVD * (hf + 1)].reshape(NKT, 128, KVD)
        wvh = np.ascontiguousarray(
            wvh.transpose(1, 0, 2).astype(bf)).reshape(128, NKT * KVD)
        woh = Wo[1024 * hf:1024 * (hf + 1), :].reshape(HPC, 128, HID)
        woh = np.ascontiguousarray(
            woh.transpose(1, 0, 2).astype(bf)).reshape(128, HPC * HID)
        halves.append((wqh, wkh, wvh, woh))

    inv_freq = 1.0 / (10000.0 ** (np.arange(0, D, 2, dtype=np.float64) / D))
    t = np.arange(S, dtype=np.float64)
    freqs = np.outer(t, inv_freq)
    emb = np.concatenate([freqs, freqs], -1)
    cosT = np.cos(emb).T
    sinTs_f = np.sin(emb).T.copy()
    sinTs_f[:64] *= -1.0
    p64 = np.zeros((D, D), dtype=np.float64)
    for d in range(D):
        p64[d, (d + 64) % D] = 1.0
    dmask = np.triu(np.ones((128, 128), dtype=np.float64))
    tbl = np.ascontiguousarray(
        np.concatenate([cosT, sinTs_f, p64, dmask], axis=1)).astype(bf)
    return xTs, halves, tbl


_CACHE = {}


def kernel(hidden_states, Wq, Wk, Wv, Wo, _trace=False, _tmpdir=None):
    hidden_states = np.ascontiguousarray(hidden_states, dtype=np.float32)
    Wq = np.ascontiguousarray(Wq, dtype=np.float32)
    Wk = np.ascontiguousarray(Wk, dtype=np.float32)
    Wv = np.ascontiguousarray(Wv, dtype=np.float32)
    Wo = np.ascontiguousarray(Wo, dtype=np.float32)

    if "nc" not in _CACHE:
        _CACHE["nc"] = build_kernel()
    nc = _CACHE["nc"]
    xTs, halves, tbl = host_prep(hidden_states, Wq, Wk, Wv, Wo)

    in_maps = []
    for cid in range(8):
        b, hf = cid // 2, cid % 2
        wqh, wkh, wvh, woh = halves[hf]
        in_maps.append({
            "xT": xTs[b], "wq8": wqh, "wk": wkh, "wv": wvh,
            "wo": woh, "tbl": tbl,
        })
    res = run_bass_kernel_spmd(nc, in_maps, list(range(8)),
                               trace=_trace, tmpdir=_tmpdir)
    out = np.zeros((B, S, HID), dtype=np.float32)
    for cid in range(8):
        out[cid // 2] += res.results[cid]["out"]
        out[cid // 2][0:640, 0:1024] += res.results[cid]["out2"]
    if _trace:
        return out, res
    return out
